# revision 24
# baseline (speedup 1.0000x reference)
"""EqualizedModulatedConv2d (StyleGAN2) Trainium2 kernel.

Strategy: data-parallel over batch B=16 across 8 NeuronCores (2 samples/core).
Each core runs the full pipeline for its samples:
  1. style FC: esT[i,b] = elr * (lin_scale * (style @ fcW.T)[b,i] + fc_bias[i])
  2. w2T[i,o] = sum_t wT[i,o,t]^2 (from f32r-rounded weights)
  3. denomT[o,b] = sum_i w2T[i,o] * esT[i,b]^2 ; normT = 1/sqrt(denom + 1e-8)
  4. xm = x * esT (per in-channel, per sample) -> rounded to f32r
  5. conv: implicit GEMM, 9 taps x 4 iC chunks accumulated in PSUM (f32r
     matmuls, free dim 512 = 8 rows x 64 cols of the 66-wide padded image)
  6. demod: out = acc * normT during PSUM->SBUF copy, then DMA out.

Host side: pads x spatially (66x66), transposes weight to [iC, oC, 9],
fc_weight to [S, iC], style to [S, B]; gathers per-core outputs.
"""
import numpy as np

B, IC, OC, K, H, W, S = 16, 512, 512, 3, 64, 64, 512
NCORES = 8
BL = B // NCORES          # samples per core
PW = W + 2                # padded width
RT = 8                    # output rows per tile
NRT = H // RT             # row tiles
ICC = IC // 128           # in-channel chunks
OCC = OC // 128           # out-channel chunks
SC = S // 128             # style-dim chunks
ELR = (2.0 / (IC * K * K)) ** 0.5
LIN = (2.0 / S) ** 0.5

_CACHE = {}


def _build():
    import concourse.bacc as bacc
    import concourse.mybir as mybir
    import concourse.tile as tile

    f32 = mybir.dt.float32
    f32r = mybir.dt.float32r

    nc = bacc.Bacc(None, target_bir_lowering=False, debug=False)
    xp = nc.dram_tensor("xp", [BL, IC, H + 2, PW], f32, kind="ExternalInput").ap()
    wt = nc.dram_tensor("wt", [IC, OC, K * K], f32r, kind="ExternalInput").ap()
    fcw = nc.dram_tensor("fcw", [S, IC], f32, kind="ExternalInput").ap()
    st = nc.dram_tensor("st", [S, BL], f32, kind="ExternalInput").ap()
    fcb = nc.dram_tensor("fcb", [IC, 1], f32, kind="ExternalInput").ap()
    y = nc.dram_tensor("y", [BL, OC, H, W], f32, kind="ExternalOutput").ap()

    with tile.TileContext(nc) as tc:
        with (
            tc.tile_pool(name="wtp", bufs=1) as wtp,
            tc.tile_pool(name="fcp", bufs=1) as fcp,
            tc.tile_pool(name="sml", bufs=1) as sml,
            tc.tile_pool(name="w2t", bufs=1) as w2t,
            tc.tile_pool(name="xin", bufs=12) as xinp,
            tc.tile_pool(name="xmp", bufs=12) as xmp,
            tc.tile_pool(name="outp", bufs=4) as outp,
            tc.tile_pool(name="acc", bufs=6, space="PSUM") as accp,
            tc.tile_pool(name="pacc", bufs=2, space="PSUM") as paccp,
        ):
            # ---- load fc params first (small, gate the style FC) ----
            st_sb = fcp.tile([128, SC, BL], f32)
            nc.sync.dma_start(st_sb[:], st.rearrange("(sc p) b -> p sc b", p=128))
            fcb_sb = fcp.tile([128, ICC], f32)
            nc.sync.dma_start(fcb_sb[:], fcb.rearrange("(ic p) z -> p (ic z)", p=128))
            fcw_r = fcw.rearrange("(sc p) i -> p sc i", p=128)
            fcw_sbs = []
            for sc in range(SC):
                fcw_chunk = fcp.tile([128, IC], f32, tag=f"fcw{sc}")
                nc.sync.dma_start(fcw_chunk[:], fcw_r[:, sc, :])
                fcw_sbs.append(fcw_chunk)

            # ---- style FC -> esT[i, b] = elr*s ----
            ebias = sml.tile([128, ICC], f32)
            nc.scalar.mul(ebias[:], fcb_sb[:], ELR)
            es_sbs, ss_sbs = [], []
            for ic in range(ICC):
                ps = paccp.tile([128, BL], f32, tag="pp")
                for sc in range(SC):
                    nc.tensor.matmul(
                        ps[:], fcw_sbs[sc][:, ic * 128:(ic + 1) * 128], st_sb[:, sc, :],
                        start=(sc == 0), stop=(sc == SC - 1),
                    )
                es_c = sml.tile([128, BL], f32, tag=f"es{ic}")
                nc.scalar.activation(
                    es_c[:], ps[:], mybir.ActivationFunctionType.Identity,
                    bias=ebias[:, ic:ic + 1], scale=ELR * LIN,
                )
                ss_c = sml.tile([128, BL], f32, tag=f"ss{ic}")
                nc.vector.tensor_mul(ss_c[:], es_c[:], es_c[:])
                es_sbs.append(es_c)
                ss_sbs.append(ss_c)

            # ---- x load + modulate helper (emit first tiles before weights
            # so their DMAs take queue precedence) ----
            xp_r = xp.rearrange("b (ic p) r c -> b ic p (r c)", p=128)
            xm_cache = {}

            def load_mod(b, rt):
                if (b, rt) in xm_cache:
                    return xm_cache.pop((b, rt))
                r0 = rt * RT
                xm = []
                for ic in range(ICC):
                    xin = xinp.tile([128, (RT + 2) * PW], f32, tag="xin")
                    nc.sync.dma_start(
                        xin[:], xp_r[b, ic, :, r0 * PW:(r0 + RT + 2) * PW]
                    )
                    xmt = xmp.tile([128, (RT + 2) * PW], f32r, tag="xm")
                    nc.vector.tensor_scalar_mul(xmt[:], xin[:], es_sbs[ic][:, b:b + 1])
                    xm.append(xmt.rearrange("p (r c) -> p r c", c=PW))
                return xm


            # ---- weights (f32r): one tile per iC chunk; DMAs oC-outer so the
            # first psum group (oc=0) gates on only the first 4 transfers.
            # w2 squares per-slice as each transfer lands. ----
            wt_r = wt.rearrange("(ic p) o t -> p ic o t", p=128)
            wt_sbs = []
            for ic in range(ICC):
                wt_chunk = wtp.tile([128, OC, K * K], f32r, tag=f"wt{ic}")
                wt_sbs.append(wt_chunk)
            w2_sb = sml.tile([128, ICC, OC], f32)

            def load_wt(ic, oc):
                sl = slice(oc * 128, (oc + 1) * 128)
                nc.sync.dma_start(
                    wt_sbs[ic][:, sl, :].rearrange("p o t -> p (o t)"),
                    wt_r[:, ic, sl, :].rearrange("p o t -> p (o t)"),
                )

            def square_wt(ic, oc):
                sl = slice(oc * 128, (oc + 1) * 128)
                sq = w2t.tile([128, 128, K * K], f32, tag="w2tmp")
                wv = wt_sbs[ic][:, sl, :].bitcast(f32)
                nc.vector.tensor_mul(sq[:], wv, wv)
                nc.vector.reduce_sum(
                    w2_sb[:, ic, sl], sq[:], axis=mybir.AxisListType.X
                )

            load_wt(0, 0)
            square_wt(0, 0)
            xm_cache[(0, 0)] = load_mod(0, 0)
            for oc in range(OCC):
                for ic in range(ICC):
                    if (ic, oc) != (0, 0):
                        load_wt(ic, oc)
                        square_wt(ic, oc)

            # ---- demod norm tiles (computed after first rt group) ----
            norm_sb = sml.tile([128, OCC, BL], f32)
            sqd = sml.tile([128, OCC, BL], f32)
            eps_sb = sml.tile([128, 1], f32)
            nc.vector.memset(eps_sb[:], 1e-8)

            def compute_norm():
                for oc in range(OCC):
                    pd = paccp.tile([128, BL], f32, tag="pp")
                    for ic in range(ICC):
                        nc.tensor.matmul(
                            pd[:], w2_sb[:, ic, oc * 128:(oc + 1) * 128],
                            ss_sbs[ic][:],
                            start=(ic == 0), stop=(ic == ICC - 1),
                        )
                    nc.scalar.activation(
                        sqd[:, oc, :], pd[:], mybir.ActivationFunctionType.Sqrt,
                        bias=eps_sb[:],
                    )
                    nc.vector.reciprocal(norm_sb[:, oc, :], sqd[:, oc, :])

            # ---- main conv loop ----
            compute_norm()
            for b in range(BL):
                for rt in range(NRT):
                    r0 = rt * RT
                    xm = load_mod(b, rt)
                    for oc in range(OCC):
                        acc = accp.tile([128, RT * W], f32)
                        first, last = (0, 0), (ICC - 1, K * K - 1)
                        for ic in range(ICC):
                            for t in range(K * K):
                                ky, kx = divmod(t, K)
                                nc.tensor.matmul(
                                    acc[:],
                                    wt_sbs[ic][:, oc * 128:(oc + 1) * 128, t],
                                    xm[ic][:, ky:ky + RT, kx:kx + W],
                                    start=((ic, t) == first), stop=((ic, t) == last),
                                )
                        ot = outp.tile([128, RT * W], f32, tag="ot")
                        y_dst = y[b, oc * 128:(oc + 1) * 128, r0:r0 + RT, :].rearrange(
                            "p r c -> p (r c)"
                        )
                        is_last = (b == BL - 1 and rt == NRT - 1 and oc == OCC - 1)
                        nslice = 2 if is_last else 1
                        step = RT * W // nslice
                        for si in range(nslice):
                            ssl = slice(si * step, (si + 1) * step)
                            nc.vector.tensor_scalar_mul(
                                ot[:, ssl], acc[:, ssl], norm_sb[:, oc, b:b + 1]
                            )
                            nc.sync.dma_start(y_dst[:, ssl], ot[:, ssl])
    nc.compile()
    return nc


class _Runner:
    """Persistent jitted PJRT executor for the SPMD kernel (axon path)."""

    def __init__(self, nc, n_cores):
        import jax
        import numpy as np
        from jax.sharding import Mesh, PartitionSpec
        try:
            from jax.experimental.shard_map import shard_map
        except ImportError:
            from jax.shard_map import shard_map
        import concourse.mybir as mybir
        from concourse.bass2jax import (
            _bass_exec_p, install_neuronx_cc_hook, partition_id_tensor,
        )

        install_neuronx_cc_hook()
        self.jax = jax
        self.n_cores = n_cores
        partition_name = (
            nc.partition_id_tensor.name if nc.partition_id_tensor else None
        )
        in_names, out_names, out_avals, zero_outs = [], [], [], []
        for alloc in nc.m.functions[0].allocations:
            if not isinstance(alloc, mybir.MemoryLocationSet):
                continue
            name = alloc.memorylocations[0].name
            if alloc.kind == "ExternalInput":
                if name != partition_name:
                    in_names.append(name)
            elif alloc.kind == "ExternalOutput":
                out_names.append(name)
                shape = tuple(alloc.tensor_shape)
                dtype = mybir.dt.np(alloc.dtype)
                out_avals.append(jax.core.ShapedArray(shape, dtype))
                zero_outs.append(np.zeros(shape, dtype))
        self.in_names, self.out_names, self.out_avals = in_names, out_names, out_avals

        def _body(*args):
            operands = list(args)
            if partition_name is not None:
                operands.append(partition_id_tensor())
            return tuple(
                _bass_exec_p.bind(
                    *operands,
                    out_avals=tuple(out_avals),
                    in_names=tuple(in_names + out_names + ([partition_name] if partition_name else [])),
                    out_names=tuple(out_names),
                    lowering_input_output_aliases=(),
                    sim_require_finite=False,
                    sim_require_nnan=False,
                    nc=nc,
                )
            )

        devices = jax.devices()[:n_cores]
        mesh = Mesh(np.asarray(devices), ("core",))
        n_params = len(in_names)
        self.fn = jax.jit(
            shard_map(
                _body, mesh=mesh,
                in_specs=(PartitionSpec("core"),) * (n_params + len(out_names)),
                out_specs=(PartitionSpec("core"),) * len(out_names),
                check_rep=False,
            ),
            keep_unused=True,
        )
        self.sharding = jax.sharding.NamedSharding(mesh, PartitionSpec("core"))
        self._dev_zeros = [
            jax.device_put(
                np.zeros((n_cores * z.shape[0], *z.shape[1:]), z.dtype), self.sharding
            )
            for z in zero_outs
        ]

    def put_inputs(self, in_maps):
        concat = [
            np.concatenate(
                [np.asarray(in_maps[c][n]) for c in range(self.n_cores)], axis=0
            )
            for n in self.in_names
        ]
        return [self.jax.device_put(a, self.sharding) for a in concat]

    def run(self, dev_args):
        outs = self.fn(*dev_args, *self._dev_zeros)
        self.jax.block_until_ready(outs)
        return outs

    def results(self, outs):
        res = []
        for c in range(self.n_cores):
            d = {}
            for i, name in enumerate(self.out_names):
                full = np.asarray(outs[i])
                d[name] = full.reshape(self.n_cores, *self.out_avals[i].shape)[c]
            res.append(d)
        return res


def _get_runner():
    if "runner" not in _CACHE:
        nc = _build()
        _CACHE["nc"] = nc
        _CACHE["runner"] = _Runner(nc, NCORES)
    return _CACHE["runner"]


def _prep_inputs(x, style, weight, fc_weight, fc_bias):
    """Host-side sharding + layout marshalling. Returns per-core input maps."""
    x = np.asarray(x, dtype=np.float32)
    style = np.asarray(style, dtype=np.float32)
    weight = np.asarray(weight, dtype=np.float32)
    fc_weight = np.asarray(fc_weight, dtype=np.float32)
    fc_bias = np.asarray(fc_bias, dtype=np.float32)

    xpad = np.zeros((B, IC, H + 2, PW), dtype=np.float32)
    xpad[:, :, 1:H + 1, 1:W + 1] = x
    wt_host = np.ascontiguousarray(
        weight.transpose(1, 0, 2, 3).reshape(IC, OC, K * K)
    )
    fcw_host = np.ascontiguousarray(fc_weight.T)
    fcb_host = np.ascontiguousarray(fc_bias.reshape(IC, 1))

    in_maps = []
    for c in range(NCORES):
        sl = slice(c * BL, (c + 1) * BL)
        in_maps.append({
            "xp": np.ascontiguousarray(xpad[sl]),
            "wt": wt_host,
            "fcw": fcw_host,
            "st": np.ascontiguousarray(style[sl].T),
            "fcb": fcb_host,
        })
    return in_maps


def kernel(x, style, weight, fc_weight, fc_bias):
    runner = _get_runner()
    in_maps = _prep_inputs(x, style, weight, fc_weight, fc_bias)
    dev_args = runner.put_inputs(in_maps)
    outs = runner.run(dev_args)
    res = runner.results(outs)
    out = np.concatenate([res[c]["y"] for c in range(NCORES)], axis=0)
    return out.astype(np.float32)


# revision 26
# speedup vs baseline: 1.0005x; 1.0005x over previous
"""EqualizedModulatedConv2d (StyleGAN2) Trainium2 kernel.

Strategy: data-parallel over batch B=16 across 8 NeuronCores (2 samples/core).
Each core runs the full pipeline for its samples:
  1. style FC: esT[i,b] = elr * (lin_scale * (style @ fcW.T)[b,i] + fc_bias[i])
  2. w2T[i,o] = sum_t wT[i,o,t]^2 (from f32r-rounded weights)
  3. denomT[o,b] = sum_i w2T[i,o] * esT[i,b]^2 ; normT = 1/sqrt(denom + 1e-8)
  4. xm = x * esT (per in-channel, per sample) -> rounded to f32r
  5. conv: implicit GEMM, 9 taps x 4 iC chunks accumulated in PSUM (f32r
     matmuls, free dim 512 = 8 rows x 64 cols of the 66-wide padded image)
  6. demod: out = acc * normT during PSUM->SBUF copy, then DMA out.

Host side: pads x spatially (66x66), transposes weight to [iC, oC, 9],
fc_weight to [S, iC], style to [S, B]; gathers per-core outputs.
"""
import numpy as np

B, IC, OC, K, H, W, S = 16, 512, 512, 3, 64, 64, 512
NCORES = 8
BL = B // NCORES          # samples per core
PW = W + 2                # padded width
RT = 8                    # output rows per tile
NRT = H // RT             # row tiles
ICC = IC // 128           # in-channel chunks
OCC = OC // 128           # out-channel chunks
SC = S // 128             # style-dim chunks
ELR = (2.0 / (IC * K * K)) ** 0.5
LIN = (2.0 / S) ** 0.5

_CACHE = {}


def _build():
    import concourse.bacc as bacc
    import concourse.mybir as mybir
    import concourse.tile as tile

    f32 = mybir.dt.float32
    f32r = mybir.dt.float32r

    nc = bacc.Bacc(None, target_bir_lowering=False, debug=False)
    xp = nc.dram_tensor("xp", [BL, IC, H + 2, PW], f32, kind="ExternalInput").ap()
    wt = nc.dram_tensor("wt", [IC, OC, K * K], f32r, kind="ExternalInput").ap()
    fcw = nc.dram_tensor("fcw", [S, IC], f32, kind="ExternalInput").ap()
    st = nc.dram_tensor("st", [S, BL], f32, kind="ExternalInput").ap()
    fcb = nc.dram_tensor("fcb", [IC, 1], f32, kind="ExternalInput").ap()
    y = nc.dram_tensor("y", [BL, OC, H, W], f32, kind="ExternalOutput").ap()

    with tile.TileContext(nc) as tc:
        with (
            tc.tile_pool(name="wtp", bufs=1) as wtp,
            tc.tile_pool(name="fcp", bufs=1) as fcp,
            tc.tile_pool(name="sml", bufs=1) as sml,
            tc.tile_pool(name="w2t", bufs=1) as w2t,
            tc.tile_pool(name="xin", bufs=12) as xinp,
            tc.tile_pool(name="xmp", bufs=12) as xmp,
            tc.tile_pool(name="outp", bufs=4) as outp,
            tc.tile_pool(name="acc", bufs=6, space="PSUM") as accp,
            tc.tile_pool(name="pacc", bufs=2, space="PSUM") as paccp,
        ):
            # ---- load fc params first (small, gate the style FC) ----
            st_sb = fcp.tile([128, SC, BL], f32)
            nc.scalar.dma_start(st_sb[:], st.rearrange("(sc p) b -> p sc b", p=128))
            fcb_sb = fcp.tile([128, ICC], f32)
            nc.scalar.dma_start(fcb_sb[:], fcb.rearrange("(ic p) z -> p (ic z)", p=128))
            fcw_r = fcw.rearrange("(sc p) i -> p sc i", p=128)
            fcw_sbs = []
            for sc in range(SC):
                fcw_chunk = fcp.tile([128, IC], f32, tag=f"fcw{sc}")
                nc.sync.dma_start(fcw_chunk[:], fcw_r[:, sc, :])
                fcw_sbs.append(fcw_chunk)

            # ---- style FC -> esT[i, b] = elr*s ----
            ebias = sml.tile([128, ICC], f32)
            nc.scalar.mul(ebias[:], fcb_sb[:], ELR)
            es_sbs, ss_sbs = [], []
            for ic in range(ICC):
                ps = paccp.tile([128, BL], f32, tag="pp")
                for sc in range(SC):
                    nc.tensor.matmul(
                        ps[:], fcw_sbs[sc][:, ic * 128:(ic + 1) * 128], st_sb[:, sc, :],
                        start=(sc == 0), stop=(sc == SC - 1),
                    )
                es_c = sml.tile([128, BL], f32, tag=f"es{ic}")
                nc.scalar.activation(
                    es_c[:], ps[:], mybir.ActivationFunctionType.Identity,
                    bias=ebias[:, ic:ic + 1], scale=ELR * LIN,
                )
                ss_c = sml.tile([128, BL], f32, tag=f"ss{ic}")
                nc.vector.tensor_mul(ss_c[:], es_c[:], es_c[:])
                es_sbs.append(es_c)
                ss_sbs.append(ss_c)

            # ---- x load + modulate helper (emit first tiles before weights
            # so their DMAs take queue precedence) ----
            xp_r = xp.rearrange("b (ic p) r c -> b ic p (r c)", p=128)
            xm_cache = {}

            def load_mod(b, rt):
                if (b, rt) in xm_cache:
                    return xm_cache.pop((b, rt))
                r0 = rt * RT
                xm = []
                for ic in range(ICC):
                    xin = xinp.tile([128, (RT + 2) * PW], f32, tag="xin")
                    nc.sync.dma_start(
                        xin[:], xp_r[b, ic, :, r0 * PW:(r0 + RT + 2) * PW]
                    )
                    xmt = xmp.tile([128, (RT + 2) * PW], f32r, tag="xm")
                    nc.vector.tensor_scalar_mul(xmt[:], xin[:], es_sbs[ic][:, b:b + 1])
                    xm.append(xmt.rearrange("p (r c) -> p r c", c=PW))
                return xm


            # ---- weights (f32r): one tile per iC chunk; DMAs oC-outer so the
            # first psum group (oc=0) gates on only the first 4 transfers.
            # w2 squares per-slice as each transfer lands. ----
            wt_r = wt.rearrange("(ic p) o t -> p ic o t", p=128)
            wt_sbs = []
            for ic in range(ICC):
                wt_chunk = wtp.tile([128, OC, K * K], f32r, tag=f"wt{ic}")
                wt_sbs.append(wt_chunk)
            w2_sb = sml.tile([128, ICC, OC], f32)

            def load_wt(ic, oc):
                sl = slice(oc * 128, (oc + 1) * 128)
                nc.sync.dma_start(
                    wt_sbs[ic][:, sl, :].rearrange("p o t -> p (o t)"),
                    wt_r[:, ic, sl, :].rearrange("p o t -> p (o t)"),
                )

            def square_wt(ic, oc):
                sl = slice(oc * 128, (oc + 1) * 128)
                sq = w2t.tile([128, 128, K * K], f32, tag="w2tmp")
                wv = wt_sbs[ic][:, sl, :].bitcast(f32)
                nc.vector.tensor_mul(sq[:], wv, wv)
                nc.vector.reduce_sum(
                    w2_sb[:, ic, sl], sq[:], axis=mybir.AxisListType.X
                )

            load_wt(0, 0)
            square_wt(0, 0)
            xm_cache[(0, 0)] = load_mod(0, 0)
            for oc in range(OCC):
                for ic in range(ICC):
                    if (ic, oc) != (0, 0):
                        load_wt(ic, oc)
                        square_wt(ic, oc)

            # ---- demod norm tiles (computed after first rt group) ----
            norm_sb = sml.tile([128, OCC, BL], f32)
            sqd = sml.tile([128, OCC, BL], f32)
            eps_sb = sml.tile([128, 1], f32)
            nc.vector.memset(eps_sb[:], 1e-8)

            def compute_norm():
                for oc in range(OCC):
                    pd = paccp.tile([128, BL], f32, tag="pp")
                    for ic in range(ICC):
                        nc.tensor.matmul(
                            pd[:], w2_sb[:, ic, oc * 128:(oc + 1) * 128],
                            ss_sbs[ic][:],
                            start=(ic == 0), stop=(ic == ICC - 1),
                        )
                    nc.scalar.activation(
                        sqd[:, oc, :], pd[:], mybir.ActivationFunctionType.Sqrt,
                        bias=eps_sb[:],
                    )
                    nc.vector.reciprocal(norm_sb[:, oc, :], sqd[:, oc, :])

            # ---- main conv loop ----
            compute_norm()
            for b in range(BL):
                for rt in range(NRT):
                    r0 = rt * RT
                    xm = load_mod(b, rt)
                    for oc in range(OCC):
                        acc = accp.tile([128, RT * W], f32)
                        first, last = (0, 0), (ICC - 1, K * K - 1)
                        for ic in range(ICC):
                            for t in range(K * K):
                                ky, kx = divmod(t, K)
                                nc.tensor.matmul(
                                    acc[:],
                                    wt_sbs[ic][:, oc * 128:(oc + 1) * 128, t],
                                    xm[ic][:, ky:ky + RT, kx:kx + W],
                                    start=((ic, t) == first), stop=((ic, t) == last),
                                )
                        ot = outp.tile([128, RT * W], f32, tag="ot")
                        y_dst = y[b, oc * 128:(oc + 1) * 128, r0:r0 + RT, :].rearrange(
                            "p r c -> p (r c)"
                        )
                        is_last = (b == BL - 1 and rt == NRT - 1 and oc == OCC - 1)
                        nslice = 2 if is_last else 1
                        step = RT * W // nslice
                        for si in range(nslice):
                            ssl = slice(si * step, (si + 1) * step)
                            nc.vector.tensor_scalar_mul(
                                ot[:, ssl], acc[:, ssl], norm_sb[:, oc, b:b + 1]
                            )
                            nc.sync.dma_start(y_dst[:, ssl], ot[:, ssl])
    nc.compile()
    return nc


class _Runner:
    """Persistent jitted PJRT executor for the SPMD kernel (axon path)."""

    def __init__(self, nc, n_cores):
        import jax
        import numpy as np
        from jax.sharding import Mesh, PartitionSpec
        try:
            from jax.experimental.shard_map import shard_map
        except ImportError:
            from jax.shard_map import shard_map
        import concourse.mybir as mybir
        from concourse.bass2jax import (
            _bass_exec_p, install_neuronx_cc_hook, partition_id_tensor,
        )

        install_neuronx_cc_hook()
        self.jax = jax
        self.n_cores = n_cores
        partition_name = (
            nc.partition_id_tensor.name if nc.partition_id_tensor else None
        )
        in_names, out_names, out_avals, zero_outs = [], [], [], []
        for alloc in nc.m.functions[0].allocations:
            if not isinstance(alloc, mybir.MemoryLocationSet):
                continue
            name = alloc.memorylocations[0].name
            if alloc.kind == "ExternalInput":
                if name != partition_name:
                    in_names.append(name)
            elif alloc.kind == "ExternalOutput":
                out_names.append(name)
                shape = tuple(alloc.tensor_shape)
                dtype = mybir.dt.np(alloc.dtype)
                out_avals.append(jax.core.ShapedArray(shape, dtype))
                zero_outs.append(np.zeros(shape, dtype))
        self.in_names, self.out_names, self.out_avals = in_names, out_names, out_avals

        def _body(*args):
            operands = list(args)
            if partition_name is not None:
                operands.append(partition_id_tensor())
            return tuple(
                _bass_exec_p.bind(
                    *operands,
                    out_avals=tuple(out_avals),
                    in_names=tuple(in_names + out_names + ([partition_name] if partition_name else [])),
                    out_names=tuple(out_names),
                    lowering_input_output_aliases=(),
                    sim_require_finite=False,
                    sim_require_nnan=False,
                    nc=nc,
                )
            )

        devices = jax.devices()[:n_cores]
        mesh = Mesh(np.asarray(devices), ("core",))
        n_params = len(in_names)
        self.fn = jax.jit(
            shard_map(
                _body, mesh=mesh,
                in_specs=(PartitionSpec("core"),) * (n_params + len(out_names)),
                out_specs=(PartitionSpec("core"),) * len(out_names),
                check_rep=False,
            ),
            keep_unused=True,
        )
        self.sharding = jax.sharding.NamedSharding(mesh, PartitionSpec("core"))
        self._dev_zeros = [
            jax.device_put(
                np.zeros((n_cores * z.shape[0], *z.shape[1:]), z.dtype), self.sharding
            )
            for z in zero_outs
        ]

    def put_inputs(self, in_maps):
        concat = [
            np.concatenate(
                [np.asarray(in_maps[c][n]) for c in range(self.n_cores)], axis=0
            )
            for n in self.in_names
        ]
        return [self.jax.device_put(a, self.sharding) for a in concat]

    def run(self, dev_args):
        outs = self.fn(*dev_args, *self._dev_zeros)
        self.jax.block_until_ready(outs)
        return outs

    def results(self, outs):
        res = []
        for c in range(self.n_cores):
            d = {}
            for i, name in enumerate(self.out_names):
                full = np.asarray(outs[i])
                d[name] = full.reshape(self.n_cores, *self.out_avals[i].shape)[c]
            res.append(d)
        return res


def _get_runner():
    if "runner" not in _CACHE:
        nc = _build()
        _CACHE["nc"] = nc
        _CACHE["runner"] = _Runner(nc, NCORES)
    return _CACHE["runner"]


def _prep_inputs(x, style, weight, fc_weight, fc_bias):
    """Host-side sharding + layout marshalling. Returns per-core input maps."""
    x = np.asarray(x, dtype=np.float32)
    style = np.asarray(style, dtype=np.float32)
    weight = np.asarray(weight, dtype=np.float32)
    fc_weight = np.asarray(fc_weight, dtype=np.float32)
    fc_bias = np.asarray(fc_bias, dtype=np.float32)

    xpad = np.zeros((B, IC, H + 2, PW), dtype=np.float32)
    xpad[:, :, 1:H + 1, 1:W + 1] = x
    wt_host = np.ascontiguousarray(
        weight.transpose(1, 0, 2, 3).reshape(IC, OC, K * K)
    )
    fcw_host = np.ascontiguousarray(fc_weight.T)
    fcb_host = np.ascontiguousarray(fc_bias.reshape(IC, 1))

    in_maps = []
    for c in range(NCORES):
        sl = slice(c * BL, (c + 1) * BL)
        in_maps.append({
            "xp": np.ascontiguousarray(xpad[sl]),
            "wt": wt_host,
            "fcw": fcw_host,
            "st": np.ascontiguousarray(style[sl].T),
            "fcb": fcb_host,
        })
    return in_maps


def kernel(x, style, weight, fc_weight, fc_bias):
    runner = _get_runner()
    in_maps = _prep_inputs(x, style, weight, fc_weight, fc_bias)
    dev_args = runner.put_inputs(in_maps)
    outs = runner.run(dev_args)
    res = runner.results(outs)
    out = np.concatenate([res[c]["y"] for c in range(NCORES)], axis=0)
    return out.astype(np.float32)


# revision 28
# speedup vs baseline: 1.3475x; 1.3469x over previous
"""EqualizedModulatedConv2d (StyleGAN2) Trainium2 kernel.

Strategy: data-parallel over batch B=16 across 8 NeuronCores (2 samples/core).
Each core runs the full pipeline for its samples:
  1. style FC: esT[i,b] = elr * (lin_scale * (style @ fcW.T)[b,i] + fc_bias[i])
  2. w2T[i,o] = sum_t wT[i,o,t]^2 (from f32r-rounded weights)
  3. denomT[o,b] = sum_i w2T[i,o] * esT[i,b]^2 ; normT = 1/sqrt(denom + 1e-8)
  4. xm = x * esT (per in-channel, per sample) -> rounded to f32r
  5. conv: implicit GEMM, 9 taps x 4 iC chunks accumulated in PSUM (f32r
     matmuls, free dim 512 = 8 rows x 64 cols of the 66-wide padded image)
  6. demod: out = acc * normT during PSUM->SBUF copy, then DMA out.

Host side: pads x spatially (66x66), transposes weight to [iC, oC, 9],
fc_weight to [S, iC], style to [S, B]; gathers per-core outputs.
"""
import numpy as np

B, IC, OC, K, H, W, S = 16, 512, 512, 3, 64, 64, 512
NCORES = 8
BL = B // NCORES          # samples per core
PW = W + 2                # padded width
RT = 8                    # output rows per tile
NRT = H // RT             # row tiles
ICC = IC // 128           # in-channel chunks
OCC = OC // 128           # out-channel chunks
SC = S // 128             # style-dim chunks
ELR = (2.0 / (IC * K * K)) ** 0.5
LIN = (2.0 / S) ** 0.5

_CACHE = {}


def _build():
    import concourse.bacc as bacc
    import concourse.mybir as mybir
    import concourse.tile as tile

    f32 = mybir.dt.float32
    f32r = mybir.dt.float32r

    nc = bacc.Bacc(None, target_bir_lowering=False, debug=False)
    xp = nc.dram_tensor("xp", [BL, IC, H + 2, PW], f32, kind="ExternalInput").ap()
    wt = nc.dram_tensor("wt", [IC, OC, K * K], f32r, kind="ExternalInput").ap()
    fcw = nc.dram_tensor("fcw", [S, IC], f32, kind="ExternalInput").ap()
    st = nc.dram_tensor("st", [S, BL], f32, kind="ExternalInput").ap()
    fcb = nc.dram_tensor("fcb", [IC, 1], f32, kind="ExternalInput").ap()
    y = nc.dram_tensor("y", [BL, OC, H, W], f32, kind="ExternalOutput").ap()

    with tile.TileContext(nc) as tc:
        with (
            tc.tile_pool(name="wtp", bufs=1) as wtp,
            tc.tile_pool(name="fcp", bufs=1) as fcp,
            tc.tile_pool(name="sml", bufs=1) as sml,
            tc.tile_pool(name="w2t", bufs=1) as w2t,
            tc.tile_pool(name="xin", bufs=12) as xinp,
            tc.tile_pool(name="xmp", bufs=12) as xmp,
            tc.tile_pool(name="outp", bufs=4) as outp,
            tc.tile_pool(name="acc", bufs=6, space="PSUM") as accp,
            tc.tile_pool(name="pacc", bufs=2, space="PSUM") as paccp,
        ):
            # ---- load fc params first (small, gate the style FC) ----
            st_sb = fcp.tile([128, SC, BL], f32)
            nc.scalar.dma_start(st_sb[:], st.rearrange("(sc p) b -> p sc b", p=128))
            fcb_sb = fcp.tile([128, ICC], f32)
            nc.scalar.dma_start(fcb_sb[:], fcb.rearrange("(ic p) z -> p (ic z)", p=128))
            fcw_r = fcw.rearrange("(sc p) i -> p sc i", p=128)
            fcw_sbs = []
            for sc in range(SC):
                fcw_chunk = fcp.tile([128, IC], f32, tag=f"fcw{sc}")
                nc.sync.dma_start(fcw_chunk[:], fcw_r[:, sc, :])
                fcw_sbs.append(fcw_chunk)

            # ---- style FC -> esT[i, b] = elr*s ----
            ebias = sml.tile([128, ICC], f32)
            nc.scalar.mul(ebias[:], fcb_sb[:], ELR)
            es_sbs, ss_sbs = [], []
            for ic in range(ICC):
                ps = paccp.tile([128, BL], f32, tag="pp")
                for sc in range(SC):
                    nc.tensor.matmul(
                        ps[:], fcw_sbs[sc][:, ic * 128:(ic + 1) * 128], st_sb[:, sc, :],
                        start=(sc == 0), stop=(sc == SC - 1),
                    )
                es_c = sml.tile([128, BL], f32, tag=f"es{ic}")
                nc.scalar.activation(
                    es_c[:], ps[:], mybir.ActivationFunctionType.Identity,
                    bias=ebias[:, ic:ic + 1], scale=ELR * LIN,
                )
                ss_c = sml.tile([128, BL], f32, tag=f"ss{ic}")
                nc.vector.tensor_mul(ss_c[:], es_c[:], es_c[:])
                es_sbs.append(es_c)
                ss_sbs.append(ss_c)

            # ---- x load + modulate helper (emit first tiles before weights
            # so their DMAs take queue precedence) ----
            xp_r = xp.rearrange("b (ic p) r c -> b ic p (r c)", p=128)
            xm_cache = {}

            def load_mod(b, rt):
                if (b, rt) in xm_cache:
                    return xm_cache.pop((b, rt))
                r0 = rt * RT
                xm = []
                for ic in range(ICC):
                    xin = xinp.tile([128, (RT + 2) * PW], f32, tag="xin")
                    nc.sync.dma_start(
                        xin[:], xp_r[b, ic, :, r0 * PW:(r0 + RT + 2) * PW]
                    )
                    xmt = xmp.tile([128, (RT + 2) * PW], f32r, tag="xm")
                    nc.vector.tensor_scalar_mul(xmt[:], xin[:], es_sbs[ic][:, b:b + 1])
                    xm.append(xmt.rearrange("p (r c) -> p r c", c=PW))
                return xm


            # ---- weights (f32r): one tile per iC chunk; DMAs oC-outer so the
            # first psum group (oc=0) gates on only the first 4 transfers.
            # w2 squares per-slice as each transfer lands. ----
            wt_r = wt.rearrange("(ic p) o t -> p ic o t", p=128)
            wt_sbs = []
            for ic in range(ICC):
                wt_chunk = wtp.tile([128, OC, K * K], f32r, tag=f"wt{ic}")
                wt_sbs.append(wt_chunk)
            w2_sb = sml.tile([128, ICC, OC], f32)

            def load_wt(ic, oc):
                sl = slice(oc * 128, (oc + 1) * 128)
                nc.sync.dma_start(
                    wt_sbs[ic][:, sl, :].rearrange("p o t -> p (o t)"),
                    wt_r[:, ic, sl, :].rearrange("p o t -> p (o t)"),
                )

            def square_wt(ic, oc):
                sl = slice(oc * 128, (oc + 1) * 128)
                sq = w2t.tile([128, 128, K * K], f32, tag="w2tmp")
                wv = wt_sbs[ic][:, sl, :].bitcast(f32)
                nc.vector.tensor_mul(sq[:], wv, wv)
                nc.vector.reduce_sum(
                    w2_sb[:, ic, sl], sq[:], axis=mybir.AxisListType.X
                )

            load_wt(0, 0)
            square_wt(0, 0)
            xm_cache[(0, 0)] = load_mod(0, 0)
            for oc in range(OCC):
                for ic in range(ICC):
                    if (ic, oc) != (0, 0):
                        load_wt(ic, oc)
                        square_wt(ic, oc)

            # ---- demod norm tiles (computed after first rt group) ----
            norm_sb = sml.tile([128, OCC, BL], f32)
            sqd = sml.tile([128, OCC, BL], f32)
            eps_sb = sml.tile([128, 1], f32)
            nc.vector.memset(eps_sb[:], 1e-8)

            def compute_norm():
                for oc in range(OCC):
                    pd = paccp.tile([128, BL], f32, tag="pp")
                    for ic in range(ICC):
                        nc.tensor.matmul(
                            pd[:], w2_sb[:, ic, oc * 128:(oc + 1) * 128],
                            ss_sbs[ic][:],
                            start=(ic == 0), stop=(ic == ICC - 1),
                        )
                    nc.scalar.activation(
                        sqd[:, oc, :], pd[:], mybir.ActivationFunctionType.Sqrt,
                        bias=eps_sb[:],
                    )
                    nc.vector.reciprocal(norm_sb[:, oc, :], sqd[:, oc, :])

            # ---- main conv loop ----
            compute_norm()
            for b in range(BL):
                for rt in range(NRT):
                    r0 = rt * RT
                    xm = load_mod(b, rt)
                    for oc in range(OCC):
                        acc = accp.tile([128, RT * W], f32)
                        first, last = (0, 0), (ICC - 1, K * K - 1)
                        for ic in range(ICC):
                            for t in range(K * K):
                                ky, kx = divmod(t, K)
                                nc.tensor.matmul(
                                    acc[:],
                                    wt_sbs[ic][:, oc * 128:(oc + 1) * 128, t],
                                    xm[ic][:, ky:ky + RT, kx:kx + W],
                                    start=((ic, t) == first), stop=((ic, t) == last),
                                )
                        ot = outp.tile([128, RT * W], f32, tag="ot")
                        y_dst = y[b, oc * 128:(oc + 1) * 128, r0:r0 + RT, :].rearrange(
                            "p r c -> p (r c)"
                        )
                        is_last = (b == BL - 1 and rt == NRT - 1 and oc == OCC - 1)
                        nslice = 2 if is_last else 1
                        step = RT * W // nslice
                        for si in range(nslice):
                            ssl = slice(si * step, (si + 1) * step)
                            nc.vector.tensor_scalar_mul(
                                ot[:, ssl], acc[:, ssl], norm_sb[:, oc, b:b + 1]
                            )
                            nc.sync.dma_start(y_dst[:, ssl], ot[:, ssl])
    nc.compile()
    return nc


class _Runner:
    """Persistent jitted PJRT executor for the SPMD kernel (axon path)."""

    def __init__(self, nc, n_cores):
        import jax
        import numpy as np
        from jax.sharding import Mesh, PartitionSpec
        try:
            from jax.experimental.shard_map import shard_map
        except ImportError:
            from jax.shard_map import shard_map
        import concourse.mybir as mybir
        from concourse.bass2jax import (
            _bass_exec_p, install_neuronx_cc_hook, partition_id_tensor,
        )

        install_neuronx_cc_hook()
        self.jax = jax
        self.n_cores = n_cores
        partition_name = (
            nc.partition_id_tensor.name if nc.partition_id_tensor else None
        )
        in_names, out_names, out_avals, zero_outs = [], [], [], []
        for alloc in nc.m.functions[0].allocations:
            if not isinstance(alloc, mybir.MemoryLocationSet):
                continue
            name = alloc.memorylocations[0].name
            if alloc.kind == "ExternalInput":
                if name != partition_name:
                    in_names.append(name)
            elif alloc.kind == "ExternalOutput":
                out_names.append(name)
                shape = tuple(alloc.tensor_shape)
                dtype = mybir.dt.np(alloc.dtype)
                out_avals.append(jax.core.ShapedArray(shape, dtype))
                zero_outs.append(np.zeros(shape, dtype))
        self.in_names, self.out_names, self.out_avals = in_names, out_names, out_avals

        def _body(*args):
            operands = list(args)
            if partition_name is not None:
                operands.append(partition_id_tensor())
            return tuple(
                _bass_exec_p.bind(
                    *operands,
                    out_avals=tuple(out_avals),
                    in_names=tuple(in_names + out_names + ([partition_name] if partition_name else [])),
                    out_names=tuple(out_names),
                    lowering_input_output_aliases=(),
                    sim_require_finite=False,
                    sim_require_nnan=False,
                    nc=nc,
                )
            )

        devices = jax.devices()[:n_cores]
        mesh = Mesh(np.asarray(devices), ("core",))
        n_params = len(in_names)
        self.fn = jax.jit(
            shard_map(
                _body, mesh=mesh,
                in_specs=(PartitionSpec("core"),) * (n_params + len(out_names)),
                out_specs=(PartitionSpec("core"),) * len(out_names),
                check_rep=False,
            ),
            keep_unused=True,
        )
        self.sharding = jax.sharding.NamedSharding(mesh, PartitionSpec("core"))
        self._dev_zeros = [
            jax.device_put(
                np.zeros((n_cores * z.shape[0], *z.shape[1:]), z.dtype), self.sharding
            )
            for z in zero_outs
        ]

    def put_inputs(self, in_maps):
        concat = [
            np.concatenate(
                [np.asarray(in_maps[c][n]) for c in range(self.n_cores)], axis=0
            )
            for n in self.in_names
        ]
        return [self.jax.device_put(a, self.sharding) for a in concat]

    def run(self, dev_args):
        outs = self.fn(*dev_args, *self._dev_zeros)
        self.jax.block_until_ready(outs)
        return outs

    def results(self, outs):
        res = []
        for c in range(self.n_cores):
            d = {}
            for i, name in enumerate(self.out_names):
                full = np.asarray(outs[i])
                d[name] = full.reshape(self.n_cores, *self.out_avals[i].shape)[c]
            res.append(d)
        return res


def _get_runner():
    if "runner" not in _CACHE:
        nc = _build()
        _CACHE["nc"] = nc
        _CACHE["runner"] = _Runner(nc, NCORES)
    return _CACHE["runner"]


def _prep_inputs(x, style, weight, fc_weight, fc_bias):
    """Host-side sharding + layout marshalling. Returns per-core input maps."""
    x = np.asarray(x, dtype=np.float32)
    style = np.asarray(style, dtype=np.float32)
    weight = np.asarray(weight, dtype=np.float32)
    fc_weight = np.asarray(fc_weight, dtype=np.float32)
    fc_bias = np.asarray(fc_bias, dtype=np.float32)

    xpad = np.zeros((B, IC, H + 2, PW), dtype=np.float32)
    xpad[:, :, 1:H + 1, 1:W + 1] = x
    wt_host = np.ascontiguousarray(
        weight.transpose(1, 0, 2, 3).reshape(IC, OC, K * K)
    )
    fcw_host = np.ascontiguousarray(fc_weight.T)
    fcb_host = np.ascontiguousarray(fc_bias.reshape(IC, 1))

    in_maps = []
    for c in range(NCORES):
        sl = slice(c * BL, (c + 1) * BL)
        in_maps.append({
            "xp": np.ascontiguousarray(xpad[sl]),
            "wt": wt_host,
            "fcw": fcw_host,
            "st": np.ascontiguousarray(style[sl].T),
            "fcb": fcb_host,
        })
    return in_maps


def kernel(x, style, weight, fc_weight, fc_bias):
    runner = _get_runner()
    in_maps = _prep_inputs(x, style, weight, fc_weight, fc_bias)
    dev_args = runner.put_inputs(in_maps)
    outs = runner.run(dev_args)
    res = runner.results(outs)
    out = np.concatenate([res[c]["y"] for c in range(NCORES)], axis=0)
    return out.astype(np.float32)


# revision 29
# speedup vs baseline: 1.3522x; 1.0035x over previous
"""EqualizedModulatedConv2d (StyleGAN2) Trainium2 kernel.

Strategy: data-parallel over batch B=16 across 8 NeuronCores (2 samples/core).
Each core runs the full pipeline for its samples:
  1. style FC: esT[i,b] = elr * (lin_scale * (style @ fcW.T)[b,i] + fc_bias[i])
  2. w2T[i,o] = sum_t wT[i,o,t]^2 (from f32r-rounded weights)
  3. denomT[o,b] = sum_i w2T[i,o] * esT[i,b]^2 ; normT = 1/sqrt(denom + 1e-8)
  4. xm = x * esT (per in-channel, per sample) -> rounded to f32r
  5. conv: implicit GEMM, 9 taps x 4 iC chunks accumulated in PSUM (f32r
     matmuls, free dim 512 = 8 rows x 64 cols of the 66-wide padded image)
  6. demod: out = acc * normT during PSUM->SBUF copy, then DMA out.

Host side: pads x spatially (66x66), transposes weight to [iC, oC, 9],
fc_weight to [S, iC], style to [S, B]; gathers per-core outputs.
"""
import numpy as np

B, IC, OC, K, H, W, S = 16, 512, 512, 3, 64, 64, 512
NCORES = 8
BL = B // NCORES          # samples per core
PW = W + 2                # padded width
RT = 8                    # output rows per tile
NRT = H // RT             # row tiles
ICC = IC // 128           # in-channel chunks
OCC = OC // 128           # out-channel chunks
SC = S // 128             # style-dim chunks
ELR = (2.0 / (IC * K * K)) ** 0.5
LIN = (2.0 / S) ** 0.5

_CACHE = {}


def _build():
    import concourse.bacc as bacc
    import concourse.mybir as mybir
    import concourse.tile as tile

    f32 = mybir.dt.float32
    f32r = mybir.dt.float32r
    ALU = mybir.AluOpType

    nc = bacc.Bacc(None, target_bir_lowering=False, debug=False)
    xp = nc.dram_tensor("xp", [BL, IC, H + 2, PW], f32, kind="ExternalInput").ap()
    wt = nc.dram_tensor("wt", [IC, OC, K * K], f32, kind="ExternalInput").ap()
    fcw = nc.dram_tensor("fcw", [S, IC], f32, kind="ExternalInput").ap()
    st = nc.dram_tensor("st", [S, BL], f32, kind="ExternalInput").ap()
    fcb = nc.dram_tensor("fcb", [IC, 1], f32, kind="ExternalInput").ap()
    y = nc.dram_tensor("y", [BL, OC, H, W], f32, kind="ExternalOutput").ap()

    TX = W // 2          # 32 winograd tiles along x
    NR = 4               # winograd taps

    with tile.TileContext(nc) as tc:
        with (
            tc.tile_pool(name="up", bufs=1) as up,
            tc.tile_pool(name="wsp", bufs=2) as wsp,
            tc.tile_pool(name="fcp", bufs=1) as fcp,
            tc.tile_pool(name="sml", bufs=1) as sml,
            tc.tile_pool(name="w2t", bufs=1) as w2t,
            tc.tile_pool(name="xin", bufs=3) as xinp,
            tc.tile_pool(name="xmp", bufs=3) as xmp,
            tc.tile_pool(name="vp", bufs=6) as vp,
            tc.tile_pool(name="itp", bufs=4) as itp,
            tc.tile_pool(name="outp", bufs=2) as outp,
            tc.tile_pool(name="acc", bufs=6, space="PSUM") as accp,
            tc.tile_pool(name="pacc", bufs=2, space="PSUM") as paccp,
        ):
            # ---- fc params ----
            st_sb = fcp.tile([128, SC, BL], f32)
            nc.scalar.dma_start(st_sb[:], st.rearrange("(sc p) b -> p sc b", p=128))
            fcb_sb = fcp.tile([128, ICC], f32)
            nc.scalar.dma_start(fcb_sb[:], fcb.rearrange("(ic p) z -> p (ic z)", p=128))
            fcw_r = fcw.rearrange("(sc p) i -> p sc i", p=128)
            fcw_sbs = []
            for sc in range(SC):
                fcw_chunk = fcp.tile([128, IC], f32, tag=f"fcw{sc}")
                nc.sync.dma_start(fcw_chunk[:], fcw_r[:, sc, :])
                fcw_sbs.append(fcw_chunk)

            # ---- style FC -> esT[i, b] = elr*s ----
            ebias = sml.tile([128, ICC], f32)
            nc.scalar.mul(ebias[:], fcb_sb[:], ELR)
            es_sbs, ss_sbs = [], []
            for ic in range(ICC):
                ps = paccp.tile([128, BL], f32, tag="pp")
                for sc in range(SC):
                    nc.tensor.matmul(
                        ps[:], fcw_sbs[sc][:, ic * 128:(ic + 1) * 128], st_sb[:, sc, :],
                        start=(sc == 0), stop=(sc == SC - 1),
                    )
                es_c = sml.tile([128, BL], f32, tag=f"es{ic}")
                nc.scalar.activation(
                    es_c[:], ps[:], mybir.ActivationFunctionType.Identity,
                    bias=ebias[:, ic:ic + 1], scale=ELR * LIN,
                )
                ss_c = sml.tile([128, BL], f32, tag=f"ss{ic}")
                nc.vector.tensor_mul(ss_c[:], es_c[:], es_c[:])
                es_sbs.append(es_c)
                ss_sbs.append(ss_c)

            # ---- x load + modulate + winograd input transform ----
            xp_r = xp.rearrange("b (ic p) r c -> b ic p (r c)", p=128)
            xm_cache = {}

            def load_v(b, rt):
                if (b, rt) in xm_cache:
                    return xm_cache.pop((b, rt))
                r0 = rt * RT
                vs = []
                for ic in range(ICC):
                    xin = xinp.tile([128, (RT + 2) * PW], f32, tag="xin")
                    nc.sync.dma_start(
                        xin[:], xp_r[b, ic, :, r0 * PW:(r0 + RT + 2) * PW]
                    )
                    xmt = xmp.tile([128, (RT + 2) * PW], f32, tag="xm")
                    nc.scalar.mul(xmt[:], xin[:], es_sbs[ic][:, b:b + 1])
                    xv = xmt.rearrange("p (r two k) -> p r two k", two=2, k=PW // 2)
                    d0 = xv[:, :, 0, 0:TX]
                    d1 = xv[:, :, 1, 0:TX]
                    d2 = xv[:, :, 0, 1:TX + 1]
                    d3 = xv[:, :, 1, 1:TX + 1]
                    vt = vp.tile([128, NR, RT + 2, TX], f32r, tag="v")
                    nc.vector.tensor_sub(vt[:, 0], d0, d2)
                    nc.vector.tensor_add(vt[:, 1], d1, d2)
                    nc.vector.tensor_sub(vt[:, 2], d2, d1)
                    nc.vector.tensor_sub(vt[:, 3], d1, d3)
                    vs.append(vt)
                return vs

            # ---- weights: stream chunks, build winograd taps u + w2 ----
            wt_r = wt.rearrange("(ic p) o t -> p ic o t", p=128)
            u_sbs = []
            for ic in range(ICC):
                u_chunk = up.tile([128, OC, K, NR], f32r, tag=f"u{ic}")
                u_sbs.append(u_chunk)
            w2_sbs = {}
            for ic in range(ICC):
                for oc in range(OCC):
                    w2s = sml.tile([128, 128], f32, tag=f"w2_{ic}_{oc}")
                    w2_sbs[(ic, oc)] = w2s

            def load_wt(ic, oc):
                sl = slice(oc * 128, (oc + 1) * 128)
                ws = wsp.tile([128, 128, K, K], f32, tag="ws")
                nc.sync.dma_start(
                    ws.rearrange("p o a b -> p (o a b)"),
                    wt_r[:, ic, sl, :].rearrange("p o t -> p (o t)"),
                )
                # w2 slice for demod norm
                sq = w2t.tile([128, 128, K * K], f32, tag="w2tmp")
                wv = ws.rearrange("p o a b -> p o (a b)")
                nc.scalar.square(sq[:], wv)
                nc.vector.reduce_sum(w2_sbs[(ic, oc)][:], sq[:],
                                     axis=mybir.AxisListType.X)
                # winograd taps: u0=w0, u1=(w0+w1+w2)/2, u2=(w0-w1+w2)/2, u3=w2
                u = u_sbs[ic]
                w0, w1, w2_ = ws[:, :, :, 0], ws[:, :, :, 1], ws[:, :, :, 2]
                nc.gpsimd.tensor_copy(u[:, sl, :, 0], w0)
                nc.gpsimd.tensor_copy(u[:, sl, :, 3], w2_)
                s02 = w2t.tile([128, 128, K], f32, tag="s02")
                nc.gpsimd.tensor_add(s02[:], w0, w2_)
                w1h = w2t.tile([128, 128, K], f32, tag="w1h")
                nc.scalar.mul(w1h[:], w1, 0.5)
                nc.vector.scalar_tensor_tensor(
                    u[:, sl, :, 1], s02[:], 0.5, w1h[:], ALU.mult, ALU.add)
                nc.vector.scalar_tensor_tensor(
                    u[:, sl, :, 2], s02[:], 0.5, w1h[:], ALU.mult, ALU.subtract)

            load_wt(0, 0)
            xm_cache[(0, 0)] = load_v(0, 0)
            for oc in range(OCC):
                for ic in range(ICC):
                    if (ic, oc) != (0, 0):
                        load_wt(ic, oc)

            # ---- demod norm: normT[o, b] (per-oc as w2 slices land) ----
            norm_sb = sml.tile([128, OCC, BL], f32)
            sqd = sml.tile([128, OCC, BL], f32)
            eps_sb = sml.tile([128, 1], f32)
            nc.vector.memset(eps_sb[:], 1e-8)
            for oc in range(OCC):
                pd = paccp.tile([128, BL], f32, tag="pp")
                for ic in range(ICC):
                    nc.tensor.matmul(
                        pd[:], w2_sbs[(ic, oc)][:], ss_sbs[ic][:],
                        start=(ic == 0), stop=(ic == ICC - 1),
                    )
                nc.scalar.activation(
                    sqd[:, oc, :], pd[:], mybir.ActivationFunctionType.Sqrt,
                    bias=eps_sb[:],
                )
                nc.vector.reciprocal(norm_sb[:, oc, :], sqd[:, oc, :])

            # ---- main winograd-conv loop ----
            for b in range(BL):
                for rt in range(NRT):
                    r0 = rt * RT
                    vs = load_v(b, rt)
                    for oc in range(OCC):
                        osl = slice(oc * 128, (oc + 1) * 128)
                        psA = accp.tile([128, 2, RT * TX], f32, tag="wacc")
                        psB = accp.tile([128, 2, RT * TX], f32, tag="wacc")
                        for r in range(NR):
                            ps = psA if r < 2 else psB
                            j = r % 2
                            for ic in range(ICC):
                                for dy in range(K):
                                    nc.tensor.matmul(
                                        ps[:, j, :],
                                        u_sbs[ic][:, osl, dy, r],
                                        vs[ic][:, r, dy:dy + RT, :],
                                        start=(ic == 0 and dy == 0),
                                        stop=(ic == ICC - 1 and dy == K - 1),
                                    )
                        # inverse transform + demod + store
                        m0, m1 = psA[:, 0, :], psA[:, 1, :]
                        m2, m3 = psB[:, 0, :], psB[:, 1, :]
                        nv = norm_sb[:, oc, b:b + 1]
                        c1 = itp.tile([128, RT * TX], f32, tag="it")
                        nc.scalar.copy(c1[:], m1)
                        a01 = itp.tile([128, RT * TX], f32, tag="it")
                        nc.vector.tensor_add(a01[:], c1[:], m0)
                        t012 = itp.tile([128, RT * TX], f32, tag="it")
                        nc.vector.tensor_add(t012[:], a01[:], m2)
                        b13 = itp.tile([128, RT * TX], f32, tag="it")
                        nc.vector.tensor_sub(b13[:], c1[:], m3)
                        t123 = itp.tile([128, RT * TX], f32, tag="it")
                        nc.vector.tensor_sub(t123[:], b13[:], m2)
                        ot = outp.tile([128, RT * W], f32, tag="ot")
                        ov = ot.rearrange("p (r k two) -> p r k two", two=2, k=TX)
                        tv0 = t012.rearrange("p (r k) -> p r k", k=TX)
                        tv1 = t123.rearrange("p (r k) -> p r k", k=TX)
                        nc.scalar.mul(ov[:, :, :, 0], tv0, nv)
                        nc.scalar.mul(ov[:, :, :, 1], tv1, nv)
                        nc.sync.dma_start(
                            y[b, osl, r0:r0 + RT, :].rearrange("p r c -> p (r c)"),
                            ot[:],
                        )
    nc.compile()
    return nc


class _Runner:
    """Persistent jitted PJRT executor for the SPMD kernel (axon path)."""

    def __init__(self, nc, n_cores):
        import jax
        import numpy as np
        from jax.sharding import Mesh, PartitionSpec
        try:
            from jax.experimental.shard_map import shard_map
        except ImportError:
            from jax.shard_map import shard_map
        import concourse.mybir as mybir
        from concourse.bass2jax import (
            _bass_exec_p, install_neuronx_cc_hook, partition_id_tensor,
        )

        install_neuronx_cc_hook()
        self.jax = jax
        self.n_cores = n_cores
        partition_name = (
            nc.partition_id_tensor.name if nc.partition_id_tensor else None
        )
        in_names, out_names, out_avals, zero_outs = [], [], [], []
        for alloc in nc.m.functions[0].allocations:
            if not isinstance(alloc, mybir.MemoryLocationSet):
                continue
            name = alloc.memorylocations[0].name
            if alloc.kind == "ExternalInput":
                if name != partition_name:
                    in_names.append(name)
            elif alloc.kind == "ExternalOutput":
                out_names.append(name)
                shape = tuple(alloc.tensor_shape)
                dtype = mybir.dt.np(alloc.dtype)
                out_avals.append(jax.core.ShapedArray(shape, dtype))
                zero_outs.append(np.zeros(shape, dtype))
        self.in_names, self.out_names, self.out_avals = in_names, out_names, out_avals

        def _body(*args):
            operands = list(args)
            if partition_name is not None:
                operands.append(partition_id_tensor())
            return tuple(
                _bass_exec_p.bind(
                    *operands,
                    out_avals=tuple(out_avals),
                    in_names=tuple(in_names + out_names + ([partition_name] if partition_name else [])),
                    out_names=tuple(out_names),
                    lowering_input_output_aliases=(),
                    sim_require_finite=False,
                    sim_require_nnan=False,
                    nc=nc,
                )
            )

        devices = jax.devices()[:n_cores]
        mesh = Mesh(np.asarray(devices), ("core",))
        n_params = len(in_names)
        self.fn = jax.jit(
            shard_map(
                _body, mesh=mesh,
                in_specs=(PartitionSpec("core"),) * (n_params + len(out_names)),
                out_specs=(PartitionSpec("core"),) * len(out_names),
                check_rep=False,
            ),
            keep_unused=True,
        )
        self.sharding = jax.sharding.NamedSharding(mesh, PartitionSpec("core"))
        self._dev_zeros = [
            jax.device_put(
                np.zeros((n_cores * z.shape[0], *z.shape[1:]), z.dtype), self.sharding
            )
            for z in zero_outs
        ]

    def put_inputs(self, in_maps):
        concat = [
            np.concatenate(
                [np.asarray(in_maps[c][n]) for c in range(self.n_cores)], axis=0
            )
            for n in self.in_names
        ]
        return [self.jax.device_put(a, self.sharding) for a in concat]

    def run(self, dev_args):
        outs = self.fn(*dev_args, *self._dev_zeros)
        self.jax.block_until_ready(outs)
        return outs

    def results(self, outs):
        res = []
        for c in range(self.n_cores):
            d = {}
            for i, name in enumerate(self.out_names):
                full = np.asarray(outs[i])
                d[name] = full.reshape(self.n_cores, *self.out_avals[i].shape)[c]
            res.append(d)
        return res


def _get_runner():
    if "runner" not in _CACHE:
        nc = _build()
        _CACHE["nc"] = nc
        _CACHE["runner"] = _Runner(nc, NCORES)
    return _CACHE["runner"]


def _prep_inputs(x, style, weight, fc_weight, fc_bias):
    """Host-side sharding + layout marshalling. Returns per-core input maps."""
    x = np.asarray(x, dtype=np.float32)
    style = np.asarray(style, dtype=np.float32)
    weight = np.asarray(weight, dtype=np.float32)
    fc_weight = np.asarray(fc_weight, dtype=np.float32)
    fc_bias = np.asarray(fc_bias, dtype=np.float32)

    xpad = np.zeros((B, IC, H + 2, PW), dtype=np.float32)
    xpad[:, :, 1:H + 1, 1:W + 1] = x
    # de-interleave columns: row layout [even cols | odd cols] so the
    # winograd input-transform reads contiguous runs
    xpad = np.ascontiguousarray(
        xpad.reshape(B, IC, H + 2, PW // 2, 2).transpose(0, 1, 2, 4, 3)
    ).reshape(B, IC, H + 2, PW)
    wt_host = np.ascontiguousarray(
        weight.transpose(1, 0, 2, 3).reshape(IC, OC, K * K)
    )
    fcw_host = np.ascontiguousarray(fc_weight.T)
    fcb_host = np.ascontiguousarray(fc_bias.reshape(IC, 1))

    in_maps = []
    for c in range(NCORES):
        sl = slice(c * BL, (c + 1) * BL)
        in_maps.append({
            "xp": np.ascontiguousarray(xpad[sl]),
            "wt": wt_host,
            "fcw": fcw_host,
            "st": np.ascontiguousarray(style[sl].T),
            "fcb": fcb_host,
        })
    return in_maps


def kernel(x, style, weight, fc_weight, fc_bias):
    runner = _get_runner()
    in_maps = _prep_inputs(x, style, weight, fc_weight, fc_bias)
    dev_args = runner.put_inputs(in_maps)
    outs = runner.run(dev_args)
    res = runner.results(outs)
    out = np.concatenate([res[c]["y"] for c in range(NCORES)], axis=0)
    return out.astype(np.float32)


# revision 30
# speedup vs baseline: 1.3553x; 1.0023x over previous
"""EqualizedModulatedConv2d (StyleGAN2) Trainium2 kernel.

Strategy: data-parallel over batch B=16 across 8 NeuronCores (2 samples/core).
Each core runs the full pipeline for its samples:
  1. style FC: esT[i,b] = elr * (lin_scale * (style @ fcW.T)[b,i] + fc_bias[i])
  2. w2T[i,o] = sum_t wT[i,o,t]^2 (from f32r-rounded weights)
  3. denomT[o,b] = sum_i w2T[i,o] * esT[i,b]^2 ; normT = 1/sqrt(denom + 1e-8)
  4. xm = x * esT (per in-channel, per sample) -> rounded to f32r
  5. conv: implicit GEMM, 9 taps x 4 iC chunks accumulated in PSUM (f32r
     matmuls, free dim 512 = 8 rows x 64 cols of the 66-wide padded image)
  6. demod: out = acc * normT during PSUM->SBUF copy, then DMA out.

Host side: pads x spatially (66x66), transposes weight to [iC, oC, 9],
fc_weight to [S, iC], style to [S, B]; gathers per-core outputs.
"""
import numpy as np

B, IC, OC, K, H, W, S = 16, 512, 512, 3, 64, 64, 512
NCORES = 8
BL = B // NCORES          # samples per core
PW = W + 2                # padded width
RT = 8                    # output rows per tile
NRT = H // RT             # row tiles
ICC = IC // 128           # in-channel chunks
OCC = OC // 128           # out-channel chunks
SC = S // 128             # style-dim chunks
ELR = (2.0 / (IC * K * K)) ** 0.5
LIN = (2.0 / S) ** 0.5

_CACHE = {}


def _build():
    import concourse.bacc as bacc
    import concourse.mybir as mybir
    import concourse.tile as tile

    f32 = mybir.dt.float32
    f32r = mybir.dt.float32r
    ALU = mybir.AluOpType

    nc = bacc.Bacc(None, target_bir_lowering=False, debug=False)
    xp = nc.dram_tensor("xp", [BL, IC, H + 2, PW], f32, kind="ExternalInput").ap()
    wt = nc.dram_tensor("wt", [IC, OC, K * K], f32, kind="ExternalInput").ap()
    fcw = nc.dram_tensor("fcw", [S, IC], f32, kind="ExternalInput").ap()
    st = nc.dram_tensor("st", [S, BL], f32, kind="ExternalInput").ap()
    fcb = nc.dram_tensor("fcb", [IC, 1], f32, kind="ExternalInput").ap()
    y = nc.dram_tensor("y", [BL, OC, H, W], f32, kind="ExternalOutput").ap()

    TX = W // 2          # 32 winograd tiles along x
    NR = 4               # winograd taps

    with tile.TileContext(nc) as tc:
        with (
            tc.tile_pool(name="up", bufs=1) as up,
            tc.tile_pool(name="wsp", bufs=2) as wsp,
            tc.tile_pool(name="fcp", bufs=1) as fcp,
            tc.tile_pool(name="sml", bufs=1) as sml,
            tc.tile_pool(name="w2t", bufs=1) as w2t,
            tc.tile_pool(name="xin", bufs=2) as xinp,
            tc.tile_pool(name="xmp", bufs=2) as xmp,
            tc.tile_pool(name="vp", bufs=8) as vp,
            tc.tile_pool(name="itp", bufs=3) as itp,
            tc.tile_pool(name="outp", bufs=2) as outp,
            tc.tile_pool(name="acc", bufs=6, space="PSUM") as accp,
            tc.tile_pool(name="pacc", bufs=2, space="PSUM") as paccp,
        ):
            # ---- fc params ----
            st_sb = fcp.tile([128, SC, BL], f32)
            nc.scalar.dma_start(st_sb[:], st.rearrange("(sc p) b -> p sc b", p=128))
            fcb_sb = fcp.tile([128, ICC], f32)
            nc.scalar.dma_start(fcb_sb[:], fcb.rearrange("(ic p) z -> p (ic z)", p=128))
            fcw_r = fcw.rearrange("(sc p) i -> p sc i", p=128)
            fcw_sbs = []
            for sc in range(SC):
                fcw_chunk = fcp.tile([128, IC], f32, tag=f"fcw{sc}")
                nc.sync.dma_start(fcw_chunk[:], fcw_r[:, sc, :])
                fcw_sbs.append(fcw_chunk)

            # ---- style FC -> esT[i, b] = elr*s ----
            ebias = sml.tile([128, ICC], f32)
            nc.scalar.mul(ebias[:], fcb_sb[:], ELR)
            es_sbs, ss_sbs = [], []
            for ic in range(ICC):
                ps = paccp.tile([128, BL], f32, tag="pp")
                for sc in range(SC):
                    nc.tensor.matmul(
                        ps[:], fcw_sbs[sc][:, ic * 128:(ic + 1) * 128], st_sb[:, sc, :],
                        start=(sc == 0), stop=(sc == SC - 1),
                    )
                es_c = sml.tile([128, BL], f32, tag=f"es{ic}")
                nc.scalar.activation(
                    es_c[:], ps[:], mybir.ActivationFunctionType.Identity,
                    bias=ebias[:, ic:ic + 1], scale=ELR * LIN,
                )
                ss_c = sml.tile([128, BL], f32, tag=f"ss{ic}")
                nc.vector.tensor_mul(ss_c[:], es_c[:], es_c[:])
                es_sbs.append(es_c)
                ss_sbs.append(ss_c)

            # ---- x load + modulate + winograd input transform ----
            xp_r = xp.rearrange("b (ic p) r c -> b ic p (r c)", p=128)
            xm_cache = {}

            def load_v(b, rt):
                if (b, rt) in xm_cache:
                    return xm_cache.pop((b, rt))
                r0 = rt * RT
                vs = []
                for ic in range(ICC):
                    xin = xinp.tile([128, (RT + 2) * PW], f32, tag="xin")
                    nc.sync.dma_start(
                        xin[:], xp_r[b, ic, :, r0 * PW:(r0 + RT + 2) * PW]
                    )
                    xmt = xmp.tile([128, (RT + 2) * PW], f32, tag="xm")
                    nc.scalar.mul(xmt[:], xin[:], es_sbs[ic][:, b:b + 1])
                    xv = xmt.rearrange("p (r two k) -> p r two k", two=2, k=PW // 2)
                    d0 = xv[:, :, 0, 0:TX]
                    d1 = xv[:, :, 1, 0:TX]
                    d2 = xv[:, :, 0, 1:TX + 1]
                    d3 = xv[:, :, 1, 1:TX + 1]
                    vt = vp.tile([128, NR, RT + 2, TX], f32r, tag="v")
                    nc.vector.tensor_sub(vt[:, 0], d0, d2)
                    nc.vector.tensor_add(vt[:, 1], d1, d2)
                    nc.vector.tensor_sub(vt[:, 2], d2, d1)
                    nc.vector.tensor_sub(vt[:, 3], d1, d3)
                    vs.append(vt)
                return vs

            # ---- weights: stream chunks, build winograd taps u + w2 ----
            wt_r = wt.rearrange("(ic p) o t -> p ic o t", p=128)
            u_sbs = []
            for ic in range(ICC):
                u_chunk = up.tile([128, OC, K, NR], f32r, tag=f"u{ic}")
                u_sbs.append(u_chunk)
            w2_sbs = {}
            for ic in range(ICC):
                for oc in range(OCC):
                    w2s = sml.tile([128, 128], f32, tag=f"w2_{ic}_{oc}")
                    w2_sbs[(ic, oc)] = w2s

            def load_wt(ic, oc):
                sl = slice(oc * 128, (oc + 1) * 128)
                ws = wsp.tile([128, 128, K, K], f32, tag="ws")
                nc.sync.dma_start(
                    ws.rearrange("p o a b -> p (o a b)"),
                    wt_r[:, ic, sl, :].rearrange("p o t -> p (o t)"),
                )
                # w2 slice for demod norm
                sq = w2t.tile([128, 128, K * K], f32, tag="w2tmp")
                wv = ws.rearrange("p o a b -> p o (a b)")
                nc.scalar.square(sq[:], wv)
                nc.vector.reduce_sum(w2_sbs[(ic, oc)][:], sq[:],
                                     axis=mybir.AxisListType.X)
                # winograd taps: u0=w0, u1=(w0+w1+w2)/2, u2=(w0-w1+w2)/2, u3=w2
                u = u_sbs[ic]
                w0, w1, w2_ = ws[:, :, :, 0], ws[:, :, :, 1], ws[:, :, :, 2]
                nc.gpsimd.tensor_copy(u[:, sl, :, 0], w0)
                nc.gpsimd.tensor_copy(u[:, sl, :, 3], w2_)
                s02 = w2t.tile([128, 128, K], f32, tag="s02")
                nc.gpsimd.tensor_add(s02[:], w0, w2_)
                w1h = w2t.tile([128, 128, K], f32, tag="w1h")
                nc.scalar.mul(w1h[:], w1, 0.5)
                nc.vector.scalar_tensor_tensor(
                    u[:, sl, :, 1], s02[:], 0.5, w1h[:], ALU.mult, ALU.add)
                nc.vector.scalar_tensor_tensor(
                    u[:, sl, :, 2], s02[:], 0.5, w1h[:], ALU.mult, ALU.subtract)

            load_wt(0, 0)
            xm_cache[(0, 0)] = load_v(0, 0)
            for ic in range(1, ICC):
                load_wt(ic, 0)
            xm_cache[(0, 1)] = load_v(0, 1)
            for oc in range(1, OCC):
                for ic in range(ICC):
                    load_wt(ic, oc)

            # ---- demod norm: normT[o, b] (per-oc as w2 slices land) ----
            norm_sb = sml.tile([128, OCC, BL], f32)
            sqd = sml.tile([128, OCC, BL], f32)
            eps_sb = sml.tile([128, 1], f32)
            nc.vector.memset(eps_sb[:], 1e-8)
            for oc in range(OCC):
                pd = paccp.tile([128, BL], f32, tag="pp")
                for ic in range(ICC):
                    nc.tensor.matmul(
                        pd[:], w2_sbs[(ic, oc)][:], ss_sbs[ic][:],
                        start=(ic == 0), stop=(ic == ICC - 1),
                    )
                nc.scalar.activation(
                    sqd[:, oc, :], pd[:], mybir.ActivationFunctionType.Sqrt,
                    bias=eps_sb[:],
                )
                nc.vector.reciprocal(norm_sb[:, oc, :], sqd[:, oc, :])

            # ---- main winograd-conv loop ----
            def conv_group(b, rt, vs, oc):
                    r0 = rt * RT
                    if True:
                        osl = slice(oc * 128, (oc + 1) * 128)
                        psA = accp.tile([128, 2, RT * TX], f32, tag="wacc")
                        psB = accp.tile([128, 2, RT * TX], f32, tag="wacc")
                        for r in range(NR):
                            ps = psA if r < 2 else psB
                            j = r % 2
                            for ic in range(ICC):
                                for dy in range(K):
                                    nc.tensor.matmul(
                                        ps[:, j, :],
                                        u_sbs[ic][:, osl, dy, r],
                                        vs[ic][:, r, dy:dy + RT, :],
                                        start=(ic == 0 and dy == 0),
                                        stop=(ic == ICC - 1 and dy == K - 1),
                                    )
                        # inverse transform + demod + store
                        m0, m1 = psA[:, 0, :], psA[:, 1, :]
                        m2, m3 = psB[:, 0, :], psB[:, 1, :]
                        nv = norm_sb[:, oc, b:b + 1]
                        c1 = itp.tile([128, RT * TX], f32, tag="it")
                        nc.scalar.copy(c1[:], m1)
                        a01 = itp.tile([128, RT * TX], f32, tag="it")
                        nc.vector.tensor_add(a01[:], c1[:], m0)
                        t012 = itp.tile([128, RT * TX], f32, tag="it")
                        nc.vector.tensor_add(t012[:], a01[:], m2)
                        b13 = itp.tile([128, RT * TX], f32, tag="it")
                        nc.vector.tensor_sub(b13[:], c1[:], m3)
                        t123 = itp.tile([128, RT * TX], f32, tag="it")
                        nc.vector.tensor_sub(t123[:], b13[:], m2)
                        ot = outp.tile([128, RT * W], f32, tag="ot")
                        ov = ot.rearrange("p (r k two) -> p r k two", two=2, k=TX)
                        tv0 = t012.rearrange("p (r k) -> p r k", k=TX)
                        tv1 = t123.rearrange("p (r k) -> p r k", k=TX)
                        nc.scalar.mul(ov[:, :, :, 0], tv0, nv)
                        nc.scalar.mul(ov[:, :, :, 1], tv1, nv)
                        nc.sync.dma_start(
                            y[b, osl, r0:r0 + RT, :].rearrange("p r c -> p (r c)"),
                            ot[:],
                        )

            # first two row-tiles of b0 interleaved oc-outer: each arriving
            # weight column-chunk enables 2 groups of PE work during the
            # initial weight stream
            vs00 = load_v(0, 0)
            vs01 = load_v(0, 1)
            for oc in range(OCC):
                conv_group(0, 0, vs00, oc)
                conv_group(0, 1, vs01, oc)
            for b in range(BL):
                for rt in range(NRT):
                    if b == 0 and rt < 2:
                        continue
                    vs = load_v(b, rt)
                    for oc in range(OCC):
                        conv_group(b, rt, vs, oc)
    nc.compile()
    return nc


class _Runner:
    """Persistent jitted PJRT executor for the SPMD kernel (axon path)."""

    def __init__(self, nc, n_cores):
        import jax
        import numpy as np
        from jax.sharding import Mesh, PartitionSpec
        try:
            from jax.experimental.shard_map import shard_map
        except ImportError:
            from jax.shard_map import shard_map
        import concourse.mybir as mybir
        from concourse.bass2jax import (
            _bass_exec_p, install_neuronx_cc_hook, partition_id_tensor,
        )

        install_neuronx_cc_hook()
        self.jax = jax
        self.n_cores = n_cores
        partition_name = (
            nc.partition_id_tensor.name if nc.partition_id_tensor else None
        )
        in_names, out_names, out_avals, zero_outs = [], [], [], []
        for alloc in nc.m.functions[0].allocations:
            if not isinstance(alloc, mybir.MemoryLocationSet):
                continue
            name = alloc.memorylocations[0].name
            if alloc.kind == "ExternalInput":
                if name != partition_name:
                    in_names.append(name)
            elif alloc.kind == "ExternalOutput":
                out_names.append(name)
                shape = tuple(alloc.tensor_shape)
                dtype = mybir.dt.np(alloc.dtype)
                out_avals.append(jax.core.ShapedArray(shape, dtype))
                zero_outs.append(np.zeros(shape, dtype))
        self.in_names, self.out_names, self.out_avals = in_names, out_names, out_avals

        def _body(*args):
            operands = list(args)
            if partition_name is not None:
                operands.append(partition_id_tensor())
            return tuple(
                _bass_exec_p.bind(
                    *operands,
                    out_avals=tuple(out_avals),
                    in_names=tuple(in_names + out_names + ([partition_name] if partition_name else [])),
                    out_names=tuple(out_names),
                    lowering_input_output_aliases=(),
                    sim_require_finite=False,
                    sim_require_nnan=False,
                    nc=nc,
                )
            )

        devices = jax.devices()[:n_cores]
        mesh = Mesh(np.asarray(devices), ("core",))
        n_params = len(in_names)
        self.fn = jax.jit(
            shard_map(
                _body, mesh=mesh,
                in_specs=(PartitionSpec("core"),) * (n_params + len(out_names)),
                out_specs=(PartitionSpec("core"),) * len(out_names),
                check_rep=False,
            ),
            keep_unused=True,
        )
        self.sharding = jax.sharding.NamedSharding(mesh, PartitionSpec("core"))
        self._dev_zeros = [
            jax.device_put(
                np.zeros((n_cores * z.shape[0], *z.shape[1:]), z.dtype), self.sharding
            )
            for z in zero_outs
        ]

    def put_inputs(self, in_maps):
        concat = [
            np.concatenate(
                [np.asarray(in_maps[c][n]) for c in range(self.n_cores)], axis=0
            )
            for n in self.in_names
        ]
        return [self.jax.device_put(a, self.sharding) for a in concat]

    def run(self, dev_args):
        outs = self.fn(*dev_args, *self._dev_zeros)
        self.jax.block_until_ready(outs)
        return outs

    def results(self, outs):
        res = []
        for c in range(self.n_cores):
            d = {}
            for i, name in enumerate(self.out_names):
                full = np.asarray(outs[i])
                d[name] = full.reshape(self.n_cores, *self.out_avals[i].shape)[c]
            res.append(d)
        return res


def _get_runner():
    if "runner" not in _CACHE:
        nc = _build()
        _CACHE["nc"] = nc
        _CACHE["runner"] = _Runner(nc, NCORES)
    return _CACHE["runner"]


def _prep_inputs(x, style, weight, fc_weight, fc_bias):
    """Host-side sharding + layout marshalling. Returns per-core input maps."""
    x = np.asarray(x, dtype=np.float32)
    style = np.asarray(style, dtype=np.float32)
    weight = np.asarray(weight, dtype=np.float32)
    fc_weight = np.asarray(fc_weight, dtype=np.float32)
    fc_bias = np.asarray(fc_bias, dtype=np.float32)

    xpad = np.zeros((B, IC, H + 2, PW), dtype=np.float32)
    xpad[:, :, 1:H + 1, 1:W + 1] = x
    # de-interleave columns: row layout [even cols | odd cols] so the
    # winograd input-transform reads contiguous runs
    xpad = np.ascontiguousarray(
        xpad.reshape(B, IC, H + 2, PW // 2, 2).transpose(0, 1, 2, 4, 3)
    ).reshape(B, IC, H + 2, PW)
    wt_host = np.ascontiguousarray(
        weight.transpose(1, 0, 2, 3).reshape(IC, OC, K * K)
    )
    fcw_host = np.ascontiguousarray(fc_weight.T)
    fcb_host = np.ascontiguousarray(fc_bias.reshape(IC, 1))

    in_maps = []
    for c in range(NCORES):
        sl = slice(c * BL, (c + 1) * BL)
        in_maps.append({
            "xp": np.ascontiguousarray(xpad[sl]),
            "wt": wt_host,
            "fcw": fcw_host,
            "st": np.ascontiguousarray(style[sl].T),
            "fcb": fcb_host,
        })
    return in_maps


def kernel(x, style, weight, fc_weight, fc_bias):
    runner = _get_runner()
    in_maps = _prep_inputs(x, style, weight, fc_weight, fc_bias)
    dev_args = runner.put_inputs(in_maps)
    outs = runner.run(dev_args)
    res = runner.results(outs)
    out = np.concatenate([res[c]["y"] for c in range(NCORES)], axis=0)
    return out.astype(np.float32)


# revision 31
# speedup vs baseline: 1.3600x; 1.0034x over previous
"""EqualizedModulatedConv2d (StyleGAN2) Trainium2 kernel.

Strategy: data-parallel over batch B=16 across 8 NeuronCores (2 samples/core).
Each core runs the full pipeline for its samples:
  1. style FC: esT[i,b] = elr * (lin_scale * (style @ fcW.T)[b,i] + fc_bias[i])
  2. w2T[i,o] = sum_t wT[i,o,t]^2 (from f32r-rounded weights)
  3. denomT[o,b] = sum_i w2T[i,o] * esT[i,b]^2 ; normT = 1/sqrt(denom + 1e-8)
  4. xm = x * esT (per in-channel, per sample) -> rounded to f32r
  5. conv: implicit GEMM, 9 taps x 4 iC chunks accumulated in PSUM (f32r
     matmuls, free dim 512 = 8 rows x 64 cols of the 66-wide padded image)
  6. demod: out = acc * normT during PSUM->SBUF copy, then DMA out.

Host side: pads x spatially (66x66), transposes weight to [iC, oC, 9],
fc_weight to [S, iC], style to [S, B]; gathers per-core outputs.
"""
import numpy as np

B, IC, OC, K, H, W, S = 16, 512, 512, 3, 64, 64, 512
NCORES = 8
BL = B // NCORES          # samples per core
PW = W + 2                # padded width
RT = 8                    # output rows per tile
NRT = H // RT             # row tiles
ICC = IC // 128           # in-channel chunks
OCC = OC // 128           # out-channel chunks
SC = S // 128             # style-dim chunks
ELR = (2.0 / (IC * K * K)) ** 0.5
LIN = (2.0 / S) ** 0.5

_CACHE = {}


def _build():
    import concourse.bacc as bacc
    import concourse.mybir as mybir
    import concourse.tile as tile

    f32 = mybir.dt.float32
    f32r = mybir.dt.float32r
    ALU = mybir.AluOpType

    nc = bacc.Bacc(None, target_bir_lowering=False, debug=False)
    xp = nc.dram_tensor("xp", [BL, IC, H + 2, PW], f32, kind="ExternalInput").ap()
    wt = nc.dram_tensor("wt", [IC, OC, K * K], f32, kind="ExternalInput").ap()
    fcw = nc.dram_tensor("fcw", [S, IC], f32, kind="ExternalInput").ap()
    st = nc.dram_tensor("st", [S, BL], f32, kind="ExternalInput").ap()
    fcb = nc.dram_tensor("fcb", [IC, 1], f32, kind="ExternalInput").ap()
    y = nc.dram_tensor("y", [BL, OC, H, W], f32, kind="ExternalOutput").ap()

    TX = W // 2          # 32 winograd tiles along x
    NR = 4               # winograd taps

    with tile.TileContext(nc) as tc:
        with (
            tc.tile_pool(name="up", bufs=1) as up,
            tc.tile_pool(name="wsp", bufs=2) as wsp,
            tc.tile_pool(name="fcp", bufs=1) as fcp,
            tc.tile_pool(name="sml", bufs=1) as sml,
            tc.tile_pool(name="w2t", bufs=1) as w2t,
            tc.tile_pool(name="xin", bufs=2) as xinp,
            tc.tile_pool(name="xmp", bufs=2) as xmp,
            tc.tile_pool(name="vp", bufs=8) as vp,
            tc.tile_pool(name="itp", bufs=3) as itp,
            tc.tile_pool(name="outp", bufs=2) as outp,
            tc.tile_pool(name="acc", bufs=6, space="PSUM") as accp,
            tc.tile_pool(name="pacc", bufs=2, space="PSUM") as paccp,
        ):
            # ---- fc params ----
            st_sb = fcp.tile([128, SC, BL], f32)
            nc.scalar.dma_start(st_sb[:], st.rearrange("(sc p) b -> p sc b", p=128))
            fcb_sb = fcp.tile([128, ICC], f32)
            nc.scalar.dma_start(fcb_sb[:], fcb.rearrange("(ic p) z -> p (ic z)", p=128))
            fcw_r = fcw.rearrange("(sc p) i -> p sc i", p=128)
            fcw_sbs = []
            for sc in range(SC):
                fcw_chunk = fcp.tile([128, IC], f32, tag=f"fcw{sc}")
                nc.sync.dma_start(fcw_chunk[:], fcw_r[:, sc, :])
                fcw_sbs.append(fcw_chunk)

            # ---- style FC -> esT[i, b] = elr*s ----
            ebias = sml.tile([128, ICC], f32)
            nc.scalar.mul(ebias[:], fcb_sb[:], ELR)
            es_sbs, ss_sbs = [], []
            for ic in range(ICC):
                ps = paccp.tile([128, BL], f32, tag="pp")
                for sc in range(SC):
                    nc.tensor.matmul(
                        ps[:], fcw_sbs[sc][:, ic * 128:(ic + 1) * 128], st_sb[:, sc, :],
                        start=(sc == 0), stop=(sc == SC - 1),
                    )
                es_c = sml.tile([128, BL], f32, tag=f"es{ic}")
                nc.scalar.activation(
                    es_c[:], ps[:], mybir.ActivationFunctionType.Identity,
                    bias=ebias[:, ic:ic + 1], scale=ELR * LIN,
                )
                ss_c = sml.tile([128, BL], f32, tag=f"ss{ic}")
                nc.vector.tensor_mul(ss_c[:], es_c[:], es_c[:])
                es_sbs.append(es_c)
                ss_sbs.append(ss_c)

            # ---- x load + modulate + winograd input transform ----
            xp_r = xp.rearrange("b (ic p) r c -> b ic p (r c)", p=128)
            xm_cache = {}

            def load_v(b, rt):
                if (b, rt) in xm_cache:
                    return xm_cache.pop((b, rt))
                r0 = rt * RT
                vs = []
                for ic in range(ICC):
                    xin = xinp.tile([128, (RT + 2) * PW], f32, tag="xin")
                    nc.sync.dma_start(
                        xin[:], xp_r[b, ic, :, r0 * PW:(r0 + RT + 2) * PW]
                    )
                    xmt = xmp.tile([128, (RT + 2) * PW], f32, tag="xm")
                    nc.scalar.mul(xmt[:], xin[:], es_sbs[ic][:, b:b + 1])
                    xv = xmt.rearrange("p (r two k) -> p r two k", two=2, k=PW // 2)
                    d0 = xv[:, :, 0, 0:TX]
                    d1 = xv[:, :, 1, 0:TX]
                    d2 = xv[:, :, 0, 1:TX + 1]
                    d3 = xv[:, :, 1, 1:TX + 1]
                    vt = vp.tile([128, NR, RT + 2, TX], f32r, tag="v")
                    nc.vector.tensor_sub(vt[:, 0], d0, d2)
                    nc.vector.tensor_add(vt[:, 1], d1, d2)
                    nc.vector.tensor_sub(vt[:, 2], d2, d1)
                    nc.vector.tensor_sub(vt[:, 3], d1, d3)
                    vs.append(vt)
                return vs

            # ---- weights: stream chunks, build winograd taps u + w2 ----
            wt_r = wt.rearrange("(ic p) o t -> p ic o t", p=128)
            u_sbs = []
            for ic in range(ICC):
                u_chunk = up.tile([128, OC, K, NR], f32r, tag=f"u{ic}")
                u_sbs.append(u_chunk)
            w2_sbs = {}
            for ic in range(ICC):
                for oc in range(OCC):
                    w2s = sml.tile([128, 128], f32, tag=f"w2_{ic}_{oc}")
                    w2_sbs[(ic, oc)] = w2s

            def load_wt(ic, oc):
                sl = slice(oc * 128, (oc + 1) * 128)
                ws = wsp.tile([128, 128, K, K], f32, tag="ws")
                nc.sync.dma_start(
                    ws.rearrange("p o a b -> p (o a b)"),
                    wt_r[:, ic, sl, :].rearrange("p o t -> p (o t)"),
                )
                # w2 slice for demod norm
                sq = w2t.tile([128, 128, K * K], f32, tag="w2tmp")
                wv = ws.rearrange("p o a b -> p o (a b)")
                nc.scalar.square(sq[:], wv)
                nc.vector.reduce_sum(w2_sbs[(ic, oc)][:], sq[:],
                                     axis=mybir.AxisListType.X)
                # winograd taps: u0=w0, u1=(w0+w1+w2)/2, u2=(w0-w1+w2)/2, u3=w2
                u = u_sbs[ic]
                w0, w1, w2_ = ws[:, :, :, 0], ws[:, :, :, 1], ws[:, :, :, 2]
                nc.gpsimd.tensor_copy(u[:, sl, :, 0], w0)
                nc.gpsimd.tensor_copy(u[:, sl, :, 3], w2_)
                s02 = w2t.tile([128, 128, K], f32, tag="s02")
                nc.gpsimd.tensor_add(s02[:], w0, w2_)
                w1h = w2t.tile([128, 128, K], f32, tag="w1h")
                nc.scalar.mul(w1h[:], w1, 0.5)
                nc.vector.scalar_tensor_tensor(
                    u[:, sl, :, 1], s02[:], 0.5, w1h[:], ALU.mult, ALU.add)
                nc.vector.scalar_tensor_tensor(
                    u[:, sl, :, 2], s02[:], 0.5, w1h[:], ALU.mult, ALU.subtract)

            load_wt(0, 0)
            xm_cache[(0, 0)] = load_v(0, 0)
            for ic in range(1, ICC):
                load_wt(ic, 0)
            xm_cache[(0, 1)] = load_v(0, 1)
            for oc in range(1, OCC):
                for ic in range(ICC):
                    load_wt(ic, oc)

            # ---- demod norm: normT[o, b] (per-oc as w2 slices land) ----
            norm_sb = sml.tile([128, OCC, BL], f32)
            sqd = sml.tile([128, OCC, BL], f32)
            eps_sb = sml.tile([128, 1], f32)
            nc.vector.memset(eps_sb[:], 1e-8)
            for oc in range(OCC):
                pd = paccp.tile([128, BL], f32, tag="pp")
                for ic in range(ICC):
                    nc.tensor.matmul(
                        pd[:], w2_sbs[(ic, oc)][:], ss_sbs[ic][:],
                        start=(ic == 0), stop=(ic == ICC - 1),
                    )
                nc.scalar.activation(
                    sqd[:, oc, :], pd[:], mybir.ActivationFunctionType.Sqrt,
                    bias=eps_sb[:],
                )
                nc.vector.reciprocal(norm_sb[:, oc, :], sqd[:, oc, :])

            # ---- main winograd-conv loop ----
            def conv_group(b, rt, vs, oc):
                    r0 = rt * RT
                    if True:
                        osl = slice(oc * 128, (oc + 1) * 128)
                        psA = accp.tile([128, 2, RT * TX], f32, tag="wacc")
                        psB = accp.tile([128, 2, RT * TX], f32, tag="wacc")
                        for r in range(NR):
                            ps = psA if r < 2 else psB
                            j = r % 2
                            for ic in range(ICC):
                                for dy in range(K):
                                    nc.tensor.matmul(
                                        ps[:, j, :],
                                        u_sbs[ic][:, osl, dy, r],
                                        vs[ic][:, r, dy:dy + RT, :],
                                        start=(ic == 0 and dy == 0),
                                        stop=(ic == ICC - 1 and dy == K - 1),
                                    )
                        # inverse transform + demod + store
                        m0, m1 = psA[:, 0, :], psA[:, 1, :]
                        m2, m3 = psB[:, 0, :], psB[:, 1, :]
                        nv = norm_sb[:, oc, b:b + 1]
                        c1 = itp.tile([128, RT * TX], f32, tag="it")
                        nc.scalar.copy(c1[:], m1)
                        a01 = itp.tile([128, RT * TX], f32, tag="it")
                        nc.vector.tensor_add(a01[:], c1[:], m0)
                        t012 = itp.tile([128, RT * TX], f32, tag="it")
                        nc.vector.tensor_add(t012[:], a01[:], m2)
                        b13 = itp.tile([128, RT * TX], f32, tag="it")
                        nc.vector.tensor_sub(b13[:], c1[:], m3)
                        t123 = itp.tile([128, RT * TX], f32, tag="it")
                        nc.vector.tensor_sub(t123[:], b13[:], m2)
                        ot = outp.tile([128, RT * W], f32, tag="ot")
                        ov = ot.rearrange("p (r k two) -> p r k two", two=2, k=TX)
                        tv0 = t012.rearrange("p (r k) -> p r k", k=TX)
                        tv1 = t123.rearrange("p (r k) -> p r k", k=TX)
                        nc.scalar.mul(ov[:, :, :, 0], tv0, nv)
                        nc.scalar.mul(ov[:, :, :, 1], tv1, nv)
                        nc.sync.dma_start(
                            y[b, osl, r0:r0 + RT, :].rearrange("p r c -> p (r c)"),
                            ot[:],
                        )

            # first two row-tiles of b0 interleaved oc-outer: each arriving
            # weight column-chunk enables 2 groups of PE work during the
            # initial weight stream
            vs00 = load_v(0, 0)
            vs01 = load_v(0, 1)
            for oc in range(2):
                conv_group(0, 0, vs00, oc)
                conv_group(0, 1, vs01, oc)
            conv_group(0, 0, vs00, 2)
            conv_group(0, 0, vs00, 3)
            conv_group(0, 1, vs01, 2)
            conv_group(0, 1, vs01, 3)
            for b in range(BL):
                for rt in range(NRT):
                    if b == 0 and rt < 2:
                        continue
                    vs = load_v(b, rt)
                    for oc in range(OCC):
                        conv_group(b, rt, vs, oc)
    nc.compile()
    return nc


class _Runner:
    """Persistent jitted PJRT executor for the SPMD kernel (axon path)."""

    def __init__(self, nc, n_cores):
        import jax
        import numpy as np
        from jax.sharding import Mesh, PartitionSpec
        try:
            from jax.experimental.shard_map import shard_map
        except ImportError:
            from jax.shard_map import shard_map
        import concourse.mybir as mybir
        from concourse.bass2jax import (
            _bass_exec_p, install_neuronx_cc_hook, partition_id_tensor,
        )

        install_neuronx_cc_hook()
        self.jax = jax
        self.n_cores = n_cores
        partition_name = (
            nc.partition_id_tensor.name if nc.partition_id_tensor else None
        )
        in_names, out_names, out_avals, zero_outs = [], [], [], []
        for alloc in nc.m.functions[0].allocations:
            if not isinstance(alloc, mybir.MemoryLocationSet):
                continue
            name = alloc.memorylocations[0].name
            if alloc.kind == "ExternalInput":
                if name != partition_name:
                    in_names.append(name)
            elif alloc.kind == "ExternalOutput":
                out_names.append(name)
                shape = tuple(alloc.tensor_shape)
                dtype = mybir.dt.np(alloc.dtype)
                out_avals.append(jax.core.ShapedArray(shape, dtype))
                zero_outs.append(np.zeros(shape, dtype))
        self.in_names, self.out_names, self.out_avals = in_names, out_names, out_avals

        def _body(*args):
            operands = list(args)
            if partition_name is not None:
                operands.append(partition_id_tensor())
            return tuple(
                _bass_exec_p.bind(
                    *operands,
                    out_avals=tuple(out_avals),
                    in_names=tuple(in_names + out_names + ([partition_name] if partition_name else [])),
                    out_names=tuple(out_names),
                    lowering_input_output_aliases=(),
                    sim_require_finite=False,
                    sim_require_nnan=False,
                    nc=nc,
                )
            )

        devices = jax.devices()[:n_cores]
        mesh = Mesh(np.asarray(devices), ("core",))
        n_params = len(in_names)
        self.fn = jax.jit(
            shard_map(
                _body, mesh=mesh,
                in_specs=(PartitionSpec("core"),) * (n_params + len(out_names)),
                out_specs=(PartitionSpec("core"),) * len(out_names),
                check_rep=False,
            ),
            keep_unused=True,
        )
        self.sharding = jax.sharding.NamedSharding(mesh, PartitionSpec("core"))
        self._dev_zeros = [
            jax.device_put(
                np.zeros((n_cores * z.shape[0], *z.shape[1:]), z.dtype), self.sharding
            )
            for z in zero_outs
        ]

    def put_inputs(self, in_maps):
        concat = [
            np.concatenate(
                [np.asarray(in_maps[c][n]) for c in range(self.n_cores)], axis=0
            )
            for n in self.in_names
        ]
        return [self.jax.device_put(a, self.sharding) for a in concat]

    def run(self, dev_args):
        outs = self.fn(*dev_args, *self._dev_zeros)
        self.jax.block_until_ready(outs)
        return outs

    def results(self, outs):
        res = []
        for c in range(self.n_cores):
            d = {}
            for i, name in enumerate(self.out_names):
                full = np.asarray(outs[i])
                d[name] = full.reshape(self.n_cores, *self.out_avals[i].shape)[c]
            res.append(d)
        return res


def _get_runner():
    if "runner" not in _CACHE:
        nc = _build()
        _CACHE["nc"] = nc
        _CACHE["runner"] = _Runner(nc, NCORES)
    return _CACHE["runner"]


def _prep_inputs(x, style, weight, fc_weight, fc_bias):
    """Host-side sharding + layout marshalling. Returns per-core input maps."""
    x = np.asarray(x, dtype=np.float32)
    style = np.asarray(style, dtype=np.float32)
    weight = np.asarray(weight, dtype=np.float32)
    fc_weight = np.asarray(fc_weight, dtype=np.float32)
    fc_bias = np.asarray(fc_bias, dtype=np.float32)

    xpad = np.zeros((B, IC, H + 2, PW), dtype=np.float32)
    xpad[:, :, 1:H + 1, 1:W + 1] = x
    # de-interleave columns: row layout [even cols | odd cols] so the
    # winograd input-transform reads contiguous runs
    xpad = np.ascontiguousarray(
        xpad.reshape(B, IC, H + 2, PW // 2, 2).transpose(0, 1, 2, 4, 3)
    ).reshape(B, IC, H + 2, PW)
    wt_host = np.ascontiguousarray(
        weight.transpose(1, 0, 2, 3).reshape(IC, OC, K * K)
    )
    fcw_host = np.ascontiguousarray(fc_weight.T)
    fcb_host = np.ascontiguousarray(fc_bias.reshape(IC, 1))

    in_maps = []
    for c in range(NCORES):
        sl = slice(c * BL, (c + 1) * BL)
        in_maps.append({
            "xp": np.ascontiguousarray(xpad[sl]),
            "wt": wt_host,
            "fcw": fcw_host,
            "st": np.ascontiguousarray(style[sl].T),
            "fcb": fcb_host,
        })
    return in_maps


def kernel(x, style, weight, fc_weight, fc_bias):
    runner = _get_runner()
    in_maps = _prep_inputs(x, style, weight, fc_weight, fc_bias)
    dev_args = runner.put_inputs(in_maps)
    outs = runner.run(dev_args)
    res = runner.results(outs)
    out = np.concatenate([res[c]["y"] for c in range(NCORES)], axis=0)
    return out.astype(np.float32)


# revision 32
# speedup vs baseline: 1.3623x; 1.0017x over previous
"""EqualizedModulatedConv2d (StyleGAN2) Trainium2 kernel.

Strategy: data-parallel over batch B=16 across 8 NeuronCores (2 samples/core).
Each core runs the full pipeline for its samples:
  1. style FC: esT[i,b] = elr * (lin_scale * (style @ fcW.T)[b,i] + fc_bias[i])
  2. w2T[i,o] = sum_t wT[i,o,t]^2 (from f32r-rounded weights)
  3. denomT[o,b] = sum_i w2T[i,o] * esT[i,b]^2 ; normT = 1/sqrt(denom + 1e-8)
  4. xm = x * esT (per in-channel, per sample) -> rounded to f32r
  5. conv: implicit GEMM, 9 taps x 4 iC chunks accumulated in PSUM (f32r
     matmuls, free dim 512 = 8 rows x 64 cols of the 66-wide padded image)
  6. demod: out = acc * normT during PSUM->SBUF copy, then DMA out.

Host side: pads x spatially (66x66), transposes weight to [iC, oC, 9],
fc_weight to [S, iC], style to [S, B]; gathers per-core outputs.
"""
import numpy as np

B, IC, OC, K, H, W, S = 16, 512, 512, 3, 64, 64, 512
NCORES = 8
BL = B // NCORES          # samples per core
PW = W + 2                # padded width
RT = 8                    # output rows per tile
NRT = H // RT             # row tiles
ICC = IC // 128           # in-channel chunks
OCC = OC // 128           # out-channel chunks
SC = S // 128             # style-dim chunks
ELR = (2.0 / (IC * K * K)) ** 0.5
LIN = (2.0 / S) ** 0.5

_CACHE = {}


def _build():
    import concourse.bacc as bacc
    import concourse.mybir as mybir
    import concourse.tile as tile

    f32 = mybir.dt.float32
    f32r = mybir.dt.float32r
    ALU = mybir.AluOpType

    nc = bacc.Bacc(None, target_bir_lowering=False, debug=False)
    xp = nc.dram_tensor("xp", [BL, IC, H + 2, PW], f32, kind="ExternalInput").ap()
    wt = nc.dram_tensor("wt", [IC, OC, K * K], f32, kind="ExternalInput").ap()
    fcw = nc.dram_tensor("fcw", [S, IC], f32, kind="ExternalInput").ap()
    st = nc.dram_tensor("st", [S, BL], f32, kind="ExternalInput").ap()
    fcb = nc.dram_tensor("fcb", [IC, 1], f32, kind="ExternalInput").ap()
    y = nc.dram_tensor("y", [BL, OC, H, W], f32, kind="ExternalOutput").ap()

    TX = W // 2          # 32 winograd tiles along x
    NR = 4               # winograd taps

    with tile.TileContext(nc) as tc:
        with (
            tc.tile_pool(name="up", bufs=1) as up,
            tc.tile_pool(name="wsp", bufs=2) as wsp,
            tc.tile_pool(name="fcp", bufs=1) as fcp,
            tc.tile_pool(name="sml", bufs=1) as sml,
            tc.tile_pool(name="w2t", bufs=1) as w2t,
            tc.tile_pool(name="xin", bufs=2) as xinp,
            tc.tile_pool(name="xmp", bufs=2) as xmp,
            tc.tile_pool(name="vp", bufs=8) as vp,
            tc.tile_pool(name="itp", bufs=3) as itp,
            tc.tile_pool(name="outp", bufs=2) as outp,
            tc.tile_pool(name="acc", bufs=6, space="PSUM") as accp,
            tc.tile_pool(name="pacc", bufs=2, space="PSUM") as paccp,
        ):
            # ---- fc params ----
            st_sb = fcp.tile([128, SC, BL], f32)
            nc.sync.dma_start(st_sb[:], st.rearrange("(sc p) b -> p sc b", p=128))
            fcb_sb = fcp.tile([128, ICC], f32)
            nc.sync.dma_start(fcb_sb[:], fcb.rearrange("(ic p) z -> p (ic z)", p=128))
            fcw_r = fcw.rearrange("(sc p) i -> p sc i", p=128)
            fcw_sbs = []
            for sc in range(SC):
                fcw_chunk = fcp.tile([128, IC], f32, tag=f"fcw{sc}")
                nc.scalar.dma_start(fcw_chunk[:], fcw_r[:, sc, :])
                fcw_sbs.append(fcw_chunk)

            # ---- style FC -> esT[i, b] = elr*s ----
            ebias = sml.tile([128, ICC], f32)
            nc.scalar.mul(ebias[:], fcb_sb[:], ELR)
            es_sbs, ss_sbs = [], []
            for ic in range(ICC):
                ps = paccp.tile([128, BL], f32, tag="pp")
                for sc in range(SC):
                    nc.tensor.matmul(
                        ps[:], fcw_sbs[sc][:, ic * 128:(ic + 1) * 128], st_sb[:, sc, :],
                        start=(sc == 0), stop=(sc == SC - 1),
                    )
                es_c = sml.tile([128, BL], f32, tag=f"es{ic}")
                nc.scalar.activation(
                    es_c[:], ps[:], mybir.ActivationFunctionType.Identity,
                    bias=ebias[:, ic:ic + 1], scale=ELR * LIN,
                )
                ss_c = sml.tile([128, BL], f32, tag=f"ss{ic}")
                nc.vector.tensor_mul(ss_c[:], es_c[:], es_c[:])
                es_sbs.append(es_c)
                ss_sbs.append(ss_c)

            # ---- x load + modulate + winograd input transform ----
            xp_r = xp.rearrange("b (ic p) r c -> b ic p (r c)", p=128)
            xm_cache = {}

            def load_v(b, rt):
                if (b, rt) in xm_cache:
                    return xm_cache.pop((b, rt))
                r0 = rt * RT
                vs = []
                for ic in range(ICC):
                    xin = xinp.tile([128, (RT + 2) * PW], f32, tag="xin")
                    nc.sync.dma_start(
                        xin[:], xp_r[b, ic, :, r0 * PW:(r0 + RT + 2) * PW]
                    )
                    xmt = xmp.tile([128, (RT + 2) * PW], f32, tag="xm")
                    nc.scalar.mul(xmt[:], xin[:], es_sbs[ic][:, b:b + 1])
                    xv = xmt.rearrange("p (r two k) -> p r two k", two=2, k=PW // 2)
                    d0 = xv[:, :, 0, 0:TX]
                    d1 = xv[:, :, 1, 0:TX]
                    d2 = xv[:, :, 0, 1:TX + 1]
                    d3 = xv[:, :, 1, 1:TX + 1]
                    vt = vp.tile([128, NR, RT + 2, TX], f32r, tag="v")
                    nc.vector.tensor_sub(vt[:, 0], d0, d2)
                    nc.vector.tensor_add(vt[:, 1], d1, d2)
                    nc.vector.tensor_sub(vt[:, 2], d2, d1)
                    nc.vector.tensor_sub(vt[:, 3], d1, d3)
                    vs.append(vt)
                return vs

            # ---- weights: stream chunks, build winograd taps u + w2 ----
            wt_r = wt.rearrange("(ic p) o t -> p ic o t", p=128)
            u_sbs = []
            for ic in range(ICC):
                u_chunk = up.tile([128, OC, K, NR], f32r, tag=f"u{ic}")
                u_sbs.append(u_chunk)
            w2_sbs = {}
            for ic in range(ICC):
                for oc in range(OCC):
                    w2s = sml.tile([128, 128], f32, tag=f"w2_{ic}_{oc}")
                    w2_sbs[(ic, oc)] = w2s

            def load_wt(ic, oc):
                sl = slice(oc * 128, (oc + 1) * 128)
                ws = wsp.tile([128, 128, K, K], f32, tag="ws")
                nc.sync.dma_start(
                    ws.rearrange("p o a b -> p (o a b)"),
                    wt_r[:, ic, sl, :].rearrange("p o t -> p (o t)"),
                )
                # w2 slice for demod norm
                sq = w2t.tile([128, 128, K * K], f32, tag="w2tmp")
                wv = ws.rearrange("p o a b -> p o (a b)")
                nc.scalar.square(sq[:], wv)
                nc.vector.reduce_sum(w2_sbs[(ic, oc)][:], sq[:],
                                     axis=mybir.AxisListType.X)
                # winograd taps: u0=w0, u1=(w0+w1+w2)/2, u2=(w0-w1+w2)/2, u3=w2
                u = u_sbs[ic]
                w0, w1, w2_ = ws[:, :, :, 0], ws[:, :, :, 1], ws[:, :, :, 2]
                nc.gpsimd.tensor_copy(u[:, sl, :, 0], w0)
                nc.gpsimd.tensor_copy(u[:, sl, :, 3], w2_)
                s02 = w2t.tile([128, 128, K], f32, tag="s02")
                nc.gpsimd.tensor_add(s02[:], w0, w2_)
                w1h = w2t.tile([128, 128, K], f32, tag="w1h")
                nc.scalar.mul(w1h[:], w1, 0.5)
                nc.vector.scalar_tensor_tensor(
                    u[:, sl, :, 1], s02[:], 0.5, w1h[:], ALU.mult, ALU.add)
                nc.vector.scalar_tensor_tensor(
                    u[:, sl, :, 2], s02[:], 0.5, w1h[:], ALU.mult, ALU.subtract)

            load_wt(0, 0)
            xm_cache[(0, 0)] = load_v(0, 0)
            for ic in range(1, ICC):
                load_wt(ic, 0)
            xm_cache[(0, 1)] = load_v(0, 1)
            for oc in range(1, OCC):
                for ic in range(ICC):
                    load_wt(ic, oc)

            # ---- demod norm: normT[o, b] (per-oc as w2 slices land) ----
            norm_sb = sml.tile([128, OCC, BL], f32)
            sqd = sml.tile([128, OCC, BL], f32)
            eps_sb = sml.tile([128, 1], f32)
            nc.vector.memset(eps_sb[:], 1e-8)
            for oc in range(OCC):
                pd = paccp.tile([128, BL], f32, tag="pp")
                for ic in range(ICC):
                    nc.tensor.matmul(
                        pd[:], w2_sbs[(ic, oc)][:], ss_sbs[ic][:],
                        start=(ic == 0), stop=(ic == ICC - 1),
                    )
                nc.scalar.activation(
                    sqd[:, oc, :], pd[:], mybir.ActivationFunctionType.Sqrt,
                    bias=eps_sb[:],
                )
                nc.vector.reciprocal(norm_sb[:, oc, :], sqd[:, oc, :])

            # ---- main winograd-conv loop ----
            def conv_group(b, rt, vs, oc):
                    r0 = rt * RT
                    if True:
                        osl = slice(oc * 128, (oc + 1) * 128)
                        psA = accp.tile([128, 2, RT * TX], f32, tag="wacc")
                        psB = accp.tile([128, 2, RT * TX], f32, tag="wacc")
                        for r in range(NR):
                            ps = psA if r < 2 else psB
                            j = r % 2
                            for ic in range(ICC):
                                for dy in range(K):
                                    nc.tensor.matmul(
                                        ps[:, j, :],
                                        u_sbs[ic][:, osl, dy, r],
                                        vs[ic][:, r, dy:dy + RT, :],
                                        start=(ic == 0 and dy == 0),
                                        stop=(ic == ICC - 1 and dy == K - 1),
                                    )
                        # inverse transform + demod + store
                        m0, m1 = psA[:, 0, :], psA[:, 1, :]
                        m2, m3 = psB[:, 0, :], psB[:, 1, :]
                        nv = norm_sb[:, oc, b:b + 1]
                        c1 = itp.tile([128, RT * TX], f32, tag="it")
                        nc.scalar.copy(c1[:], m1)
                        a01 = itp.tile([128, RT * TX], f32, tag="it")
                        nc.vector.tensor_add(a01[:], c1[:], m0)
                        t012 = itp.tile([128, RT * TX], f32, tag="it")
                        nc.vector.tensor_add(t012[:], a01[:], m2)
                        b13 = itp.tile([128, RT * TX], f32, tag="it")
                        nc.vector.tensor_sub(b13[:], c1[:], m3)
                        t123 = itp.tile([128, RT * TX], f32, tag="it")
                        nc.vector.tensor_sub(t123[:], b13[:], m2)
                        ot = outp.tile([128, RT * W], f32, tag="ot")
                        ov = ot.rearrange("p (r k two) -> p r k two", two=2, k=TX)
                        tv0 = t012.rearrange("p (r k) -> p r k", k=TX)
                        tv1 = t123.rearrange("p (r k) -> p r k", k=TX)
                        nc.scalar.mul(ov[:, :, :, 0], tv0, nv)
                        nc.scalar.mul(ov[:, :, :, 1], tv1, nv)
                        nc.sync.dma_start(
                            y[b, osl, r0:r0 + RT, :].rearrange("p r c -> p (r c)"),
                            ot[:],
                        )

            # first two row-tiles of b0 interleaved oc-outer: each arriving
            # weight column-chunk enables 2 groups of PE work during the
            # initial weight stream
            vs00 = load_v(0, 0)
            vs01 = load_v(0, 1)
            for oc in range(2):
                conv_group(0, 0, vs00, oc)
                conv_group(0, 1, vs01, oc)
            conv_group(0, 0, vs00, 2)
            conv_group(0, 0, vs00, 3)
            conv_group(0, 1, vs01, 2)
            conv_group(0, 1, vs01, 3)
            for b in range(BL):
                for rt in range(NRT):
                    if b == 0 and rt < 2:
                        continue
                    vs = load_v(b, rt)
                    for oc in range(OCC):
                        conv_group(b, rt, vs, oc)
    nc.compile()
    return nc


class _Runner:
    """Persistent jitted PJRT executor for the SPMD kernel (axon path)."""

    def __init__(self, nc, n_cores):
        import jax
        import numpy as np
        from jax.sharding import Mesh, PartitionSpec
        try:
            from jax.experimental.shard_map import shard_map
        except ImportError:
            from jax.shard_map import shard_map
        import concourse.mybir as mybir
        from concourse.bass2jax import (
            _bass_exec_p, install_neuronx_cc_hook, partition_id_tensor,
        )

        install_neuronx_cc_hook()
        self.jax = jax
        self.n_cores = n_cores
        partition_name = (
            nc.partition_id_tensor.name if nc.partition_id_tensor else None
        )
        in_names, out_names, out_avals, zero_outs = [], [], [], []
        for alloc in nc.m.functions[0].allocations:
            if not isinstance(alloc, mybir.MemoryLocationSet):
                continue
            name = alloc.memorylocations[0].name
            if alloc.kind == "ExternalInput":
                if name != partition_name:
                    in_names.append(name)
            elif alloc.kind == "ExternalOutput":
                out_names.append(name)
                shape = tuple(alloc.tensor_shape)
                dtype = mybir.dt.np(alloc.dtype)
                out_avals.append(jax.core.ShapedArray(shape, dtype))
                zero_outs.append(np.zeros(shape, dtype))
        self.in_names, self.out_names, self.out_avals = in_names, out_names, out_avals

        def _body(*args):
            operands = list(args)
            if partition_name is not None:
                operands.append(partition_id_tensor())
            return tuple(
                _bass_exec_p.bind(
                    *operands,
                    out_avals=tuple(out_avals),
                    in_names=tuple(in_names + out_names + ([partition_name] if partition_name else [])),
                    out_names=tuple(out_names),
                    lowering_input_output_aliases=(),
                    sim_require_finite=False,
                    sim_require_nnan=False,
                    nc=nc,
                )
            )

        devices = jax.devices()[:n_cores]
        mesh = Mesh(np.asarray(devices), ("core",))
        n_params = len(in_names)
        self.fn = jax.jit(
            shard_map(
                _body, mesh=mesh,
                in_specs=(PartitionSpec("core"),) * (n_params + len(out_names)),
                out_specs=(PartitionSpec("core"),) * len(out_names),
                check_rep=False,
            ),
            keep_unused=True,
        )
        self.sharding = jax.sharding.NamedSharding(mesh, PartitionSpec("core"))
        self._dev_zeros = [
            jax.device_put(
                np.zeros((n_cores * z.shape[0], *z.shape[1:]), z.dtype), self.sharding
            )
            for z in zero_outs
        ]

    def put_inputs(self, in_maps):
        concat = [
            np.concatenate(
                [np.asarray(in_maps[c][n]) for c in range(self.n_cores)], axis=0
            )
            for n in self.in_names
        ]
        return [self.jax.device_put(a, self.sharding) for a in concat]

    def run(self, dev_args):
        outs = self.fn(*dev_args, *self._dev_zeros)
        self.jax.block_until_ready(outs)
        return outs

    def results(self, outs):
        res = []
        for c in range(self.n_cores):
            d = {}
            for i, name in enumerate(self.out_names):
                full = np.asarray(outs[i])
                d[name] = full.reshape(self.n_cores, *self.out_avals[i].shape)[c]
            res.append(d)
        return res


def _get_runner():
    if "runner" not in _CACHE:
        nc = _build()
        _CACHE["nc"] = nc
        _CACHE["runner"] = _Runner(nc, NCORES)
    return _CACHE["runner"]


def _prep_inputs(x, style, weight, fc_weight, fc_bias):
    """Host-side sharding + layout marshalling. Returns per-core input maps."""
    x = np.asarray(x, dtype=np.float32)
    style = np.asarray(style, dtype=np.float32)
    weight = np.asarray(weight, dtype=np.float32)
    fc_weight = np.asarray(fc_weight, dtype=np.float32)
    fc_bias = np.asarray(fc_bias, dtype=np.float32)

    xpad = np.zeros((B, IC, H + 2, PW), dtype=np.float32)
    xpad[:, :, 1:H + 1, 1:W + 1] = x
    # de-interleave columns: row layout [even cols | odd cols] so the
    # winograd input-transform reads contiguous runs
    xpad = np.ascontiguousarray(
        xpad.reshape(B, IC, H + 2, PW // 2, 2).transpose(0, 1, 2, 4, 3)
    ).reshape(B, IC, H + 2, PW)
    wt_host = np.ascontiguousarray(
        weight.transpose(1, 0, 2, 3).reshape(IC, OC, K * K)
    )
    fcw_host = np.ascontiguousarray(fc_weight.T)
    fcb_host = np.ascontiguousarray(fc_bias.reshape(IC, 1))

    in_maps = []
    for c in range(NCORES):
        sl = slice(c * BL, (c + 1) * BL)
        in_maps.append({
            "xp": np.ascontiguousarray(xpad[sl]),
            "wt": wt_host,
            "fcw": fcw_host,
            "st": np.ascontiguousarray(style[sl].T),
            "fcb": fcb_host,
        })
    return in_maps


def kernel(x, style, weight, fc_weight, fc_bias):
    runner = _get_runner()
    in_maps = _prep_inputs(x, style, weight, fc_weight, fc_bias)
    dev_args = runner.put_inputs(in_maps)
    outs = runner.run(dev_args)
    res = runner.results(outs)
    out = np.concatenate([res[c]["y"] for c in range(NCORES)], axis=0)
    return out.astype(np.float32)
